# revision 19
# baseline (speedup 1.0000x reference)
"""NerveNet MLP critic network — Trainium2 Bass kernel (8-core data parallel).

Layout strategy: everything runs feature-major (features on SBUF partitions,
batch on the free axis), so every GEMM is `out = W.T @ xT` with the weight as
the PE stationary operand and the transposed activations streaming.  Inputs
are transposed on the host; the output is transposed back on the host.

Matmuls run as float32r (full fp32 storage, 1 cycle/row on the PE for free
dims >= 256).  Swish eviction PSUM->SBUF is fused into ScalarE activation
instructions spanning multiple PSUM banks; layer biases ride the activation
bias port (they are all zero in this problem, which enables merged
multi-module activations).

The tanh-Normal log-likelihood math is deferred to a single post-loop phase
so the ScalarE activation-table set only switches once (silu/tanh set ->
ln/exp set).
"""

import os
import sys
import types
import math

sys.path.insert(0, "/opt/trn_rl_repo")

import numpy as np

# ---------------------------------------------------------------------------
# antenv.axon_hooks shim: this container image lacks the module, which is the
# only thing standing between run_bass_kernel_spmd(trace=True) and NTFF
# profiling.  Register the ctypes-based hook from trn_agent_boot.
# ---------------------------------------------------------------------------
if "antenv.axon_hooks" not in sys.modules:
    try:
        import antenv  # noqa: F401

        _mod = types.ModuleType("antenv.axon_hooks")
        _hook_box = [None]
        _mod.set_axon_ntff_profile_hook = lambda h: _hook_box.__setitem__(0, h)
        _mod.get_axon_ntff_profile_hook = lambda: _hook_box[0]
        sys.modules["antenv.axon_hooks"] = _mod
        from trn_agent_boot.trn_boot import _ntff_profile_via_ctypes

        _mod.set_axon_ntff_profile_hook(
            _ntff_profile_via_ctypes("/opt/axon/libaxon_pjrt.so")
        )
    except Exception:
        pass

import concourse.bacc as bacc
import concourse.mybir as mybir
from concourse.tile import TileContext
from concourse.bass_utils import run_bass_kernel_spmd

F32 = mybir.dt.float32
F32R = mybir.dt.float32r
AF = mybir.ActivationFunctionType
ALU = mybir.AluOpType

# ---------------------------------------------------------------------------
# Problem constants (hardcoded per the task contract)
# ---------------------------------------------------------------------------
MODULES = ["root", "torso", "head", "arm_L", "arm_R", "hand_L", "hand_R",
           "leg_L", "leg_R", "foot_L", "foot_R"]
OBS = {"root": 60, "torso": 30, "head": 20, "arm_L": 25, "arm_R": 25,
       "hand_L": 15, "hand_R": 15, "leg_L": 25, "leg_R": 25,
       "foot_L": 15, "foot_R": 15}
ACTD = {"root": 6, "torso": 3, "head": 3, "arm_L": 4, "arm_R": 4,
        "hand_L": 5, "hand_R": 5, "leg_L": 4, "leg_R": 4,
        "foot_L": 3, "foot_R": 3}
RK = ["tracking", "control", "alive"]
B = 32768
H = 256
MIN_STD = 0.1
ENT_W = 0.01
MOTOR_SCALE = 1.0
TOTAL_OBS = 270
TOTAL_ACT = 44

N_CORES = 8
BS = B // N_CORES          # 4096 rows per core
NT = 512                   # batch-tile (free dim per matmul)
NTILES = BS // NT          # 8

LOG2PI = float(np.log(2.0 * np.pi))

# obs concat row offsets (MODULES order)
OBS_OFF = {}
_o = 0
for _k in MODULES:
    OBS_OFF[_k] = _o
    _o += OBS[_k]

ACT_OFF = {}
_o = 0
for _k in MODULES:
    ACT_OFF[_k] = _o
    _o += ACTD[_k]

# Obs rows are scattered into 3 padded 128-row chunks with every module's
# rows starting at a PE-quadrant-legal partition offset (matmul stationary /
# moving operands must start at 0/32/64/96 depending on K).  Padding rows are
# zero in both the obs and the critic L1 weight, so full-128 K-chunks stay
# exact for the critic while per-module slices drive the input layers.
CHUNKS = [(("root", 0), ("torso", 64)),
          (("head", 0), ("arm_L", 32), ("arm_R", 64)),
          (("hand_L", 0), ("hand_R", 32), ("leg_L", 64)),
          (("leg_R", 0), ("foot_L", 32), ("foot_R", 64))]
N_CHUNKS = len(CHUNKS)
PAD_OBS = N_CHUNKS * 128  # 384
MOD_CHUNK = {}
for _ci, _c in enumerate(CHUNKS):
    for _m, _slot in _c:
        assert _slot + OBS[_m] <= 128
        MOD_CHUNK[_m] = (_ci, _slot)

NONROOT = [m for m in MODULES if m != "root"]  # 10 modules with aff/eff mats
MSG_IDX = {}  # (kind, module) -> matrix slot in w_msg
for _i, _m in enumerate(NONROOT):
    MSG_IDX[("aff", _m)] = _i
    MSG_IDX[("eff", _m)] = 10 + _i

MIDX = {m: i for i, m in enumerate(MODULES)}


def _build_nc(zero_bias: bool):
    nc = bacc.Bacc(None, target_bir_lowering=False)

    # ---- DRAM parameters -------------------------------------------------
    obsT = nc.declare_dram_parameter("obsT", [PAD_OBS, BS], F32, isOutput=False)
    rawT = nc.declare_dram_parameter("rawT", [TOTAL_ACT, BS], F32, isOutput=False)
    w_inp = nc.declare_dram_parameter("w_inp", [PAD_OBS, H], F32, isOutput=False)
    w_msg = nc.declare_dram_parameter("w_msg", [128, 20 * 512], F32, isOutput=False)
    w_mot = nc.declare_dram_parameter("w_mot", [128, 22 * 108], F32, isOutput=False)
    w_e1 = nc.declare_dram_parameter("w_e1", [PAD_OBS, 512], F32, isOutput=False)
    w_e2 = nc.declare_dram_parameter("w_e2", [128, 2048], F32, isOutput=False)
    w_e3 = nc.declare_dram_parameter("w_e3", [128, 1024], F32, isOutput=False)
    w_hd = nc.declare_dram_parameter("w_hd", [128, 6], F32, isOutput=False)
    b_mot = nc.declare_dram_parameter("b_mot", [108, 1], F32, isOutput=False)
    b_hd = nc.declare_dram_parameter("b_hd", [3, 1], F32, isOutput=False)
    if not zero_bias:
        b_inp = nc.declare_dram_parameter("b_inp", [128, 22], F32, isOutput=False)
        b_msg = nc.declare_dram_parameter("b_msg", [128, 40], F32, isOutput=False)
        b_e1 = nc.declare_dram_parameter("b_e1", [128, 4], F32, isOutput=False)
        b_e2 = nc.declare_dram_parameter("b_e2", [128, 4], F32, isOutput=False)
        b_e3 = nc.declare_dram_parameter("b_e3", [128, 2], F32, isOutput=False)

    out_act = nc.declare_dram_parameter("out_act", [TOTAL_ACT, BS], F32, isOutput=True)
    out_ll = nc.declare_dram_parameter("out_ll", [1, BS], F32, isOutput=True)
    out_vals = nc.declare_dram_parameter("out_vals", [3, BS], F32, isOutput=True)
    out_ent = nc.declare_dram_parameter("out_ent", [TOTAL_ACT, 1], F32, isOutput=True)

    with TileContext(nc) as tc:
        with (
            tc.tile_pool(name="spool", bufs=1) as spool,
            tc.tile_pool(name="ppool", bufs=2, space="PSUM") as ppool,
        ):
            # ---- batch-lifetime staging tiles ---------------------------
            # mean rows 0:44, std_p rows 64:108 (partition bases must be
            # 32-aligned for engine access patterns)
            mstd_sb = spool.tile([108, BS], F32, tag="mstd")
            # values on rows 0:3, loglik on row 32 (32-aligned partition base)
            vl_sb = spool.tile([33, BS], F32, tag="vl")
            entp_sb = spool.tile([TOTAL_ACT, 1], F32, tag="entp")
            ones_sb = spool.tile([TOTAL_ACT, 1], F32, tag="ones")
            nc.vector.memset(ones_sb[:], 1.0)
            c_minstd = spool.tile([TOTAL_ACT, 1], F32, tag="cmin")
            nc.vector.memset(c_minstd[:], MIN_STD)
            c_eps = spool.tile([TOTAL_ACT, 1], F32, tag="ceps")
            nc.vector.memset(c_eps[:], 1.0 + 1e-6)

            def mm(out, lhsT, rhs, start, stop):
                nc.tensor.matmul(out, lhsT.bitcast(F32R), rhs.bitcast(F32R),
                                 start=start, stop=stop)

            # ================= phase A: network + motor ==================
            with (
                tc.tile_pool(name="wpool", bufs=1) as wpool,
                tc.tile_pool(name="obspool", bufs=2) as obspool,
                tc.tile_pool(name="xpool", bufs=1) as xpool,
                tc.tile_pool(name="tpool", bufs=2) as tpool,
                tc.tile_pool(name="hpool", bufs=1) as hpool,
            ):
                # ---- persistent weight tiles ----------------------------
                w_inp_sb = []
                w_e1_sb = []
                for ci in range(N_CHUNKS):
                    ti = wpool.tile([128, H], F32, tag=f"winp{ci}")
                    nc.sync.dma_start(out=ti[:].bitcast(F32R),
                                      in_=w_inp[ci * 128:(ci + 1) * 128, :].bitcast(F32R))
                    w_inp_sb.append(ti)
                w_msg_sb = wpool.tile([128, 20 * 512], F32, tag="wmsg")
                for q in range(4):
                    nc.sync.dma_start(out=w_msg_sb[:, q * 2560:(q + 1) * 2560].bitcast(F32R),
                                      in_=w_msg[:, q * 2560:(q + 1) * 2560].bitcast(F32R))
                w_mot_sb = wpool.tile([128, 22 * 108], F32, tag="wmot")
                nc.sync.dma_start(out=w_mot_sb[:].bitcast(F32R), in_=w_mot[:].bitcast(F32R))
                for ci in range(N_CHUNKS):
                    te = wpool.tile([128, 512], F32, tag=f"we1{ci}")
                    nc.sync.dma_start(out=te[:].bitcast(F32R),
                                      in_=w_e1[ci * 128:(ci + 1) * 128, :].bitcast(F32R))
                    w_e1_sb.append(te)
                w_e2_sb = wpool.tile([128, 2048], F32, tag="we2")
                nc.sync.dma_start(out=w_e2_sb[:].bitcast(F32R), in_=w_e2[:].bitcast(F32R))
                w_e3_sb = wpool.tile([128, 1024], F32, tag="we3")
                nc.sync.dma_start(out=w_e3_sb[:].bitcast(F32R), in_=w_e3[:].bitcast(F32R))
                w_hd_sb = wpool.tile([128, 6], F32, tag="whd")
                nc.sync.dma_start(out=w_hd_sb[:].bitcast(F32R), in_=w_hd[:].bitcast(F32R))
                b_mot_sb = wpool.tile([108, 1], F32, tag="bmot")
                nc.sync.dma_start(out=b_mot_sb[:], in_=b_mot[:])
                b_hd_sb = wpool.tile([3, 1], F32, tag="bhd")
                nc.sync.dma_start(out=b_hd_sb[:], in_=b_hd[:])
                if not zero_bias:
                    b_inp_sb = wpool.tile([128, 22], F32, tag="binp")
                    nc.sync.dma_start(out=b_inp_sb[:], in_=b_inp[:])
                    b_msg_sb = wpool.tile([128, 40], F32, tag="bmsg")
                    nc.sync.dma_start(out=b_msg_sb[:], in_=b_msg[:])
                    b_e1_sb = wpool.tile([128, 4], F32, tag="be1")
                    nc.sync.dma_start(out=b_e1_sb[:], in_=b_e1[:])
                    b_e2_sb = wpool.tile([128, 4], F32, tag="be2")
                    nc.sync.dma_start(out=b_e2_sb[:], in_=b_e2[:])
                    b_e3_sb = wpool.tile([128, 2], F32, tag="be3")
                    nc.sync.dma_start(out=b_e3_sb[:], in_=b_e3[:])

                # xall: module k's activations at cols [k*2*NT, (k+1)*2*NT);
                # within a module: cols 0:NT = features 0:128, NT:2NT = 128:256
                xall = xpool.tile([128, 11 * 2 * NT], F32, tag="xall")

                def xsl(k, kc=None):
                    base = MIDX[k] * 2 * NT
                    if kc is None:
                        return xall[:, base:base + 2 * NT]
                    return xall[:, base + kc * NT: base + (kc + 1) * NT]

                # swish-evict a psum region into dst (SBUF), optional biases
                def evict_swish(ps_ap, dst_ap, nblk, bias_cols):
                    if zero_bias:
                        nc.scalar.activation(dst_ap.bitcast(F32R), ps_ap, AF.Silu)
                    else:
                        for i in range(nblk):
                            bt, bc = bias_cols[i]
                            nc.scalar.activation(
                                dst_ap[:, i * NT:(i + 1) * NT].bitcast(F32R),
                                ps_ap[:, i * NT:(i + 1) * NT],
                                AF.Silu, bias=bt[:, bc:bc + 1])

                for t in range(NTILES):
                    ct = t * NT

                    # -- load obs chunks ----------------------------------
                    obs_t = []
                    for ci in range(N_CHUNKS):
                        ob = obspool.tile([128, NT], F32, tag=f"obs{ci}")
                        nc.sync.dma_start(
                            out=ob[:].bitcast(F32R),
                            in_=obsT[ci * 128:(ci + 1) * 128, ct:ct + NT].bitcast(F32R))
                        obs_t.append(ob)

                    # -- input layers: pairs of adjacent modules ----------
                    pairs = [("root", "torso"), ("head", "arm_L"),
                             ("arm_R", "hand_L"), ("hand_R", "leg_L"),
                             ("leg_R", "foot_L"), ("foot_R",)]
                    for pr in pairs:
                        pw = len(pr) * 2 * NT
                        ps = ppool.tile([128, 2048], F32, tag="ps")
                        for j, k in enumerate(pr):
                            ci, ro = MOD_CHUNK[k]
                            d = OBS[k]
                            for mc in range(2):
                                mm(ps[:, j * 2 * NT + mc * NT:
                                      j * 2 * NT + (mc + 1) * NT],
                                   w_inp_sb[ci][ro:ro + d, mc * 128:(mc + 1) * 128],
                                   obs_t[ci][ro:ro + d, :],
                                   start=True, stop=True)
                        base = MIDX[pr[0]] * 2 * NT
                        bias_cols = []
                        if not zero_bias:
                            for k in pr:
                                for mc in range(2):
                                    bias_cols.append((b_inp_sb, MIDX[k] * 2 + mc))
                        evict_swish(ps[:, 0:pw], xall[:, base:base + pw],
                                    len(pr) * 2, bias_cols)

                    # -- message-passing pair: swish(W.T @ src) -> tmp ----
                    def msg_pair(kind, mods, srcs):
                        ps = ppool.tile([128, 2048], F32, tag="ps")
                        for j, (k, src) in enumerate(zip(mods, srcs)):
                            mi = MSG_IDX[(kind, k)]
                            wb = mi * 512
                            for mc in range(2):
                                for kc in range(2):
                                    mm(ps[:, j * 2 * NT + mc * NT:
                                          j * 2 * NT + (mc + 1) * NT],
                                       w_msg_sb[:, wb + kc * 256 + mc * 128:
                                                wb + kc * 256 + (mc + 1) * 128],
                                       src[:, kc * NT:(kc + 1) * NT],
                                       start=(kc == 0), stop=(kc == 1))
                        tmp = tpool.tile([128, 2048], F32, tag="msgtmp")
                        w = len(mods) * 2 * NT
                        bias_cols = []
                        if not zero_bias:
                            for k in mods:
                                mi = MSG_IDX[(kind, k)]
                                for mc in range(2):
                                    bias_cols.append((b_msg_sb, mi * 2 + mc))
                        evict_swish(ps[:, 0:w], tmp[:, 0:w], len(mods) * 2, bias_cols)
                        return tmp

                    # afferent leaf -> mid (merged adds over adjacent dst)
                    tmp = msg_pair("aff", ["hand_L", "hand_R"],
                                   [xsl("hand_L"), xsl("hand_R")])
                    d0 = MIDX["arm_L"] * 2 * NT
                    nc.vector.tensor_add(xall[:, d0:d0 + 4 * NT].bitcast(F32R),
                                         xall[:, d0:d0 + 4 * NT], tmp[:, 0:4 * NT])
                    tmp = msg_pair("aff", ["foot_L", "foot_R"],
                                   [xsl("foot_L"), xsl("foot_R")])
                    d0 = MIDX["leg_L"] * 2 * NT
                    nc.vector.tensor_add(xall[:, d0:d0 + 4 * NT].bitcast(F32R),
                                         xall[:, d0:d0 + 4 * NT], tmp[:, 0:4 * NT])
                    # afferent mid -> root
                    for mods in (["arm_L", "arm_R"], ["leg_L", "leg_R"],
                                 ["torso", "head"]):
                        tmp = msg_pair("aff", mods, [xsl(mods[0]), xsl(mods[1])])
                        for j in range(2):
                            nc.vector.tensor_add(xsl("root").bitcast(F32R), xsl("root"),
                                                 tmp[:, j * 2 * NT:(j + 1) * 2 * NT])

                    # efferent root wave
                    for mods in (["torso", "head"], ["arm_L", "arm_R"],
                                 ["leg_L", "leg_R"]):
                        tmp = msg_pair("eff", mods, [xsl("root"), xsl("root")])
                        d0 = MIDX[mods[0]] * 2 * NT
                        nc.vector.tensor_add(xall[:, d0:d0 + 4 * NT].bitcast(F32R),
                                             xall[:, d0:d0 + 4 * NT], tmp[:, 0:4 * NT])
                    # efferent leaf wave
                    tmp = msg_pair("eff", ["hand_L", "hand_R"],
                                   [xsl("arm_L"), xsl("arm_R")])
                    d0 = MIDX["hand_L"] * 2 * NT
                    nc.vector.tensor_add(xall[:, d0:d0 + 4 * NT].bitcast(F32R),
                                         xall[:, d0:d0 + 4 * NT], tmp[:, 0:4 * NT])
                    tmp = msg_pair("eff", ["foot_L", "foot_R"],
                                   [xsl("leg_L"), xsl("leg_R")])
                    d0 = MIDX["foot_L"] * 2 * NT
                    nc.vector.tensor_add(xall[:, d0:d0 + 4 * NT].bitcast(F32R),
                                         xall[:, d0:d0 + 4 * NT], tmp[:, 0:4 * NT])

                    # -- motor heads: 22 accumulating matmuls -> [88, NT] -
                    psm = ppool.tile([128, 2048], F32, tag="ps")
                    nmm = 0
                    for k in MODULES:
                        for kc in range(2):
                            blk = (2 * MIDX[k] + kc) * 108
                            mm(psm[0:108, 0:NT], w_mot_sb[:, blk:blk + 108],
                               xsl(k, kc), start=(nmm == 0), stop=(nmm == 21))
                            nmm += 1
                    nc.vector.tensor_scalar_add(mstd_sb[:, ct:ct + NT],
                                                psm[0:108, 0:NT], b_mot_sb[0:108, 0:1])

                    # -- critic encoder -----------------------------------
                    ps1 = ppool.tile([128, 2048], F32, tag="ps")
                    for mc in range(4):
                        for ci in range(N_CHUNKS):
                            mm(ps1[:, mc * NT:(mc + 1) * NT],
                               w_e1_sb[ci][:, mc * 128:(mc + 1) * 128],
                               obs_t[ci][:],
                               start=(ci == 0), stop=(ci == N_CHUNKS - 1))
                    h1 = hpool.tile([128, 2048], F32, tag="h1")
                    if zero_bias:
                        nc.scalar.activation(h1[:].bitcast(F32R), ps1[:], AF.Silu)
                    else:
                        for mc in range(4):
                            nc.scalar.activation(h1[:, mc * NT:(mc + 1) * NT].bitcast(F32R),
                                                 ps1[:, mc * NT:(mc + 1) * NT],
                                                 AF.Silu, bias=b_e1_sb[:, mc:mc + 1])

                    ps2 = ppool.tile([128, 2048], F32, tag="ps")
                    for mc in range(4):
                        for kc in range(4):
                            mm(ps2[:, mc * NT:(mc + 1) * NT],
                               w_e2_sb[:, kc * 512 + mc * 128:
                                       kc * 512 + (mc + 1) * 128],
                               h1[:, kc * NT:(kc + 1) * NT],
                               start=(kc == 0), stop=(kc == 3))
                    h2 = hpool.tile([128, 2048], F32, tag="h2")
                    if zero_bias:
                        nc.scalar.activation(h2[:].bitcast(F32R), ps2[:], AF.Silu)
                    else:
                        for mc in range(4):
                            nc.scalar.activation(h2[:, mc * NT:(mc + 1) * NT].bitcast(F32R),
                                                 ps2[:, mc * NT:(mc + 1) * NT],
                                                 AF.Silu, bias=b_e2_sb[:, mc:mc + 1])

                    ps3 = ppool.tile([128, 2048], F32, tag="ps")
                    for mc in range(2):
                        for kc in range(4):
                            mm(ps3[:, mc * NT:(mc + 1) * NT],
                               w_e3_sb[:, kc * 256 + mc * 128:
                                       kc * 256 + (mc + 1) * 128],
                               h2[:, kc * NT:(kc + 1) * NT],
                               start=(kc == 0), stop=(kc == 3))
                    h3 = hpool.tile([128, 1024], F32, tag="h3")
                    if zero_bias:
                        nc.scalar.activation(h3[:].bitcast(F32R), ps3[:, 0:1024], AF.Silu)
                    else:
                        for mc in range(2):
                            nc.scalar.activation(h3[:, mc * NT:(mc + 1) * NT].bitcast(F32R),
                                                 ps3[:, mc * NT:(mc + 1) * NT],
                                                 AF.Silu, bias=b_e3_sb[:, mc:mc + 1])

                    psv = ppool.tile([128, 2048], F32, tag="ps")
                    for kc in range(2):
                        mm(psv[0:3, 0:NT], w_hd_sb[:, kc * 3:(kc + 1) * 3],
                           h3[:, kc * NT:(kc + 1) * NT],
                           start=(kc == 0), stop=(kc == 1))
                    nc.vector.tensor_scalar_add(vl_sb[0:3, ct:ct + NT],
                                                psv[0:3, 0:NT], b_hd_sb[0:3, 0:1])

            # ================= phase B: distribution math ================
            with tc.tile_pool(name="dpool", bufs=1) as dpool:
                rawT_sb = dpool.tile([TOTAL_ACT, BS], F32, tag="rawT")
                nc.sync.dma_start(out=rawT_sb[:], in_=rawT[:])
                act_sb = dpool.tile([TOTAL_ACT, BS], F32, tag="act")
                # Tanh shares the silu table set -> schedule before Ln/Exp
                nc.scalar.activation(act_sb[:], rawT_sb[:], AF.Tanh)

                mean = mstd_sb[0:44, :]
                stdp = mstd_sb[64:108, :]
                bufA = dpool.tile([TOTAL_ACT, BS], F32, tag="bufA")
                bufB = dpool.tile([TOTAL_ACT, BS], F32, tag="bufB")
                bufC = dpool.tile([TOTAL_ACT, BS], F32, tag="bufC")
                bufD = dpool.tile([TOTAL_ACT, BS], F32, tag="bufD")

                nc.scalar.activation(bufA[:], stdp, AF.Exp)                 # e^stdp
                nc.scalar.activation(bufB[:], bufA[:], AF.Ln, bias=1.0)     # softplus
                nc.scalar.activation(bufA[:], bufB[:], AF.Ln, bias=c_minstd[:],
                                     accum_out=entp_sb[:])                  # logstd
                nc.scalar.activation(bufB[:], bufA[:], AF.Exp, scale=-2.0)  # 1/std^2
                nc.vector.tensor_sub(bufC[:], rawT_sb[:], mean)             # d
                nc.vector.tensor_mul(bufD[:], bufC[:], bufC[:])             # d^2
                nc.vector.tensor_mul(bufC[:], bufD[:], bufB[:])             # z^2
                # s = -0.5*z^2 - logstd
                nc.vector.scalar_tensor_tensor(bufD[:], bufC[:], -0.5, bufA[:],
                                               op0=ALU.mult, op1=ALU.subtract)
                nc.vector.tensor_mul(bufC[:], act_sb[:], act_sb[:])         # act^2
                nc.scalar.activation(bufA[:], bufC[:], AF.Ln,
                                     scale=-1.0, bias=c_eps[:])             # tterm
                nc.vector.tensor_sub(bufB[:], bufD[:], bufA[:])             # lp

                # loglik = colsum(lp) - 22*log(2pi) via ones-matmul.
                # Plain fp32 matmul (4 cyc/row but exact products) — only 8 of them.
                for t in range(NTILES):
                    ct = t * NT
                    pl = ppool.tile([128, 2048], F32, tag="ps")
                    nc.tensor.matmul(pl[0:1, 0:NT], ones_sb[:], bufB[:, ct:ct + NT],
                                     start=True, stop=True)
                    nc.vector.tensor_scalar_add(vl_sb[32:33, ct:ct + NT], pl[0:1, 0:NT],
                                                float(-22.0 * LOG2PI))

                # ---- output DMAs ----------------------------------------
                nc.sync.dma_start(out=out_act[:], in_=act_sb[:])
                nc.sync.dma_start(out=out_ll[:], in_=vl_sb[32:33, :])
                nc.sync.dma_start(out=out_vals[:], in_=vl_sb[0:3, :])
                nc.sync.dma_start(out=out_ent[:], in_=entp_sb[:])

    nc.finalize()
    return nc


# ---------------------------------------------------------------------------
# Host-side weight packing
# ---------------------------------------------------------------------------

def _pack_msg(Ws):
    """20 [256,256] matrices -> [128, 20*512]; (kc,mc) block at j*512+kc*256+mc*128."""
    out = np.zeros((128, len(Ws) * 512), np.float32)
    for j, W in enumerate(Ws):
        blk = W.reshape(2, 128, 2, 128).transpose(1, 0, 2, 3).reshape(128, 512)
        out[:, j * 512:(j + 1) * 512] = blk
    return out


def _pack_e2(W):  # [512, 512] -> [128, 4*512]
    return np.ascontiguousarray(
        W.reshape(4, 128, 512).transpose(1, 0, 2).reshape(128, 2048))


def _pack_e3(W):  # [512, 256] -> [128, 4*256]
    return np.ascontiguousarray(
        W.reshape(4, 128, 256).transpose(1, 0, 2).reshape(128, 1024))


def _pack_hd(W):  # [256, 3] -> [128, 2*3]
    out = np.zeros((128, 6), np.float32)
    for kc in range(2):
        out[:, kc * 3:(kc + 1) * 3] = W[kc * 128:(kc + 1) * 128, :]
    return out


def _pack_mot(params):
    """Block-diagonal motor weights: [128, 22*108].

    K-chunk (module k, kc) at cols (2*MIDX[k]+kc)*108; within the 108 columns,
    module k's mean cols land at ACT_OFF[k].. and std cols at 64+ACT_OFF[k]..
    (row 64 start keeps the std block partition-base 32-aligned on chip).
    """
    out = np.zeros((128, 22 * 108), np.float32)
    for k in MODULES:
        W = np.asarray(params["motor"][k][0], np.float32) * MOTOR_SCALE  # [256, 2a]
        a = ACTD[k]
        for kc in range(2):
            blk = W[kc * 128:(kc + 1) * 128, :]
            base = (2 * MIDX[k] + kc) * 108
            out[:, base + ACT_OFF[k]: base + ACT_OFF[k] + a] = blk[:, :a]
            out[:, base + 64 + ACT_OFF[k]: base + 64 + ACT_OFF[k] + a] = blk[:, a:]
    return out


_NC_CACHE = {}
LAST_RESULT = None


def kernel(obs, raw_action, params):
    global LAST_RESULT
    obs = {k: np.asarray(v, np.float32) for k, v in obs.items()}
    raw = {k: np.asarray(v, np.float32) for k, v in raw_action.items()}

    def P(x):
        return np.asarray(x, np.float32)

    zero_bias = all([
        all(not P(params["inp"][k][1]).any() for k in MODULES),
        all(not P(params["aff"][k][1]).any() for k in NONROOT),
        all(not P(params["eff"][k][1]).any() for k in NONROOT),
        all(not P(p[1]).any() for p in params["enc"]),
    ])

    if zero_bias not in _NC_CACHE:
        _NC_CACHE[zero_bias] = _build_nc(zero_bias)
    nc = _NC_CACHE[zero_bias]

    # ---- pack host arrays ------------------------------------------------
    # padded, chunk-slot obs layout [384, B]
    obsT = np.zeros((PAD_OBS, B), np.float32)
    for k in MODULES:
        ci, slot = MOD_CHUNK[k]
        obsT[ci * 128 + slot: ci * 128 + slot + OBS[k], :] = obs[k].T
    rawT = np.ascontiguousarray(
        np.concatenate([raw[k] for k in MODULES], axis=1).T)        # [44, B]

    w_inp = np.zeros((PAD_OBS, H), np.float32)
    for k in MODULES:
        ci, slot = MOD_CHUNK[k]
        w_inp[ci * 128 + slot: ci * 128 + slot + OBS[k], :] = P(params["inp"][k][0])
    msg_mats = [P(params["aff"][k][0]) for k in NONROOT] + \
               [P(params["eff"][k][0]) for k in NONROOT]
    w_msg = _pack_msg(msg_mats)
    w_mot = _pack_mot(params)
    # critic L1 weight scattered to the padded obs-row layout [384, 512]
    w_e1_orig = P(params["enc"][0][0])                               # [270, 512]
    w_e1 = np.zeros((PAD_OBS, 512), np.float32)
    for k in MODULES:
        ci, slot = MOD_CHUNK[k]
        w_e1[ci * 128 + slot: ci * 128 + slot + OBS[k], :] = \
            w_e1_orig[OBS_OFF[k]:OBS_OFF[k] + OBS[k], :]
    w_e2 = _pack_e2(P(params["enc"][1][0]))
    w_e3 = _pack_e3(P(params["enc"][2][0]))
    w_hd = _pack_hd(np.concatenate([P(params["heads"][r][0]) for r in RK], axis=1))

    b_mot = np.zeros((108, 1), np.float32)
    for k in MODULES:
        bv = P(params["motor"][k][1]) * MOTOR_SCALE
        a = ACTD[k]
        b_mot[ACT_OFF[k]:ACT_OFF[k] + a, 0] = bv[:a]
        b_mot[64 + ACT_OFF[k]:64 + ACT_OFF[k] + a, 0] = bv[a:]
    b_hd = np.stack([P(params["heads"][r][1]) for r in RK]).reshape(3, 1)

    shared = {
        "w_inp": w_inp, "w_msg": w_msg, "w_mot": w_mot, "w_e1": w_e1,
        "w_e2": w_e2, "w_e3": w_e3, "w_hd": w_hd, "b_mot": b_mot, "b_hd": b_hd,
    }
    if not zero_bias:
        b_inp = np.zeros((128, 22), np.float32)
        for k in MODULES:
            bv = P(params["inp"][k][1])
            for mc in range(2):
                b_inp[:, MIDX[k] * 2 + mc] = bv[mc * 128:(mc + 1) * 128]
        b_msg = np.zeros((128, 40), np.float32)
        for kind in ("aff", "eff"):
            for k in NONROOT:
                mi = MSG_IDX[(kind, k)]
                bv = P(params[kind][k][1])
                for mc in range(2):
                    b_msg[:, mi * 2 + mc] = bv[mc * 128:(mc + 1) * 128]
        b_e1 = P(params["enc"][0][1]).reshape(4, 128).T.copy()
        b_e2 = P(params["enc"][1][1]).reshape(4, 128).T.copy()
        b_e3 = P(params["enc"][2][1]).reshape(2, 128).T.copy()
        shared.update({"b_inp": b_inp, "b_msg": b_msg, "b_e1": b_e1,
                       "b_e2": b_e2, "b_e3": b_e3})

    in_maps = []
    for i in range(N_CORES):
        m = dict(shared)
        m["obsT"] = np.ascontiguousarray(obsT[:, i * BS:(i + 1) * BS])
        m["rawT"] = np.ascontiguousarray(rawT[:, i * BS:(i + 1) * BS])
        in_maps.append(m)

    trace = bool(int(os.environ.get("KERNEL_TRACE", "0")))
    res = run_bass_kernel_spmd(nc, in_maps, list(range(N_CORES)), trace=trace)
    LAST_RESULT = res

    out = np.empty((B, 49), np.float32)
    ent_sum = 0.0
    for i in range(N_CORES):
        r = res.results[i]
        sl = slice(i * BS, (i + 1) * BS)
        out[sl, 0:44] = r["out_act"].T
        out[sl, 44] = r["out_ll"][0]
        out[sl, 45:48] = r["out_vals"].T
        ent_sum += float(r["out_ent"].sum())
    ent = 22.0 * (1.0 + LOG2PI) + ent_sum / B
    out[:, 48] = -ENT_W * ent
    return out


# revision 20
# speedup vs baseline: 1.1777x; 1.1777x over previous
"""NerveNet MLP critic network — Trainium2 Bass kernel (8-core data parallel).

Layout strategy: everything runs feature-major (features on SBUF partitions,
batch on the free axis), so every GEMM is `out = W.T @ xT` with the weight as
the PE stationary operand and the transposed activations streaming.  Inputs
are transposed on the host; the output is transposed back on the host.

GEMM operands are bf16 (weight loads hide under FWL, matmuls issue
back-to-back at N cycles); all accumulation is fp32 in PSUM, and the entire
tanh-Normal log-likelihood / entropy path runs in fp32.  Swish eviction
PSUM->SBUF is fused into ScalarE activation instructions spanning 4 PSUM
banks.  The ln/exp distribution math is deferred to a single post-loop phase
so the ScalarE activation-table set only switches once.
"""

import os
import sys
import types
import math

sys.path.insert(0, "/opt/trn_rl_repo")

import numpy as np
import ml_dtypes

BF = ml_dtypes.bfloat16

# ---------------------------------------------------------------------------
# antenv.axon_hooks shim (enables NTFF profiling under axon in this image)
# ---------------------------------------------------------------------------
if "antenv.axon_hooks" not in sys.modules:
    try:
        import antenv  # noqa: F401

        _mod = types.ModuleType("antenv.axon_hooks")
        _hook_box = [None]
        _mod.set_axon_ntff_profile_hook = lambda h: _hook_box.__setitem__(0, h)
        _mod.get_axon_ntff_profile_hook = lambda: _hook_box[0]
        sys.modules["antenv.axon_hooks"] = _mod
        from trn_agent_boot.trn_boot import _ntff_profile_via_ctypes

        _mod.set_axon_ntff_profile_hook(
            _ntff_profile_via_ctypes("/opt/axon/libaxon_pjrt.so")
        )
    except Exception:
        pass

import concourse.bacc as bacc
import concourse.mybir as mybir
from concourse.tile import TileContext
from concourse.bass_utils import run_bass_kernel_spmd

F32 = mybir.dt.float32
BF16 = mybir.dt.bfloat16
AF = mybir.ActivationFunctionType
ALU = mybir.AluOpType

# ---------------------------------------------------------------------------
# Problem constants (hardcoded per the task contract)
# ---------------------------------------------------------------------------
MODULES = ["root", "torso", "head", "arm_L", "arm_R", "hand_L", "hand_R",
           "leg_L", "leg_R", "foot_L", "foot_R"]
OBS = {"root": 60, "torso": 30, "head": 20, "arm_L": 25, "arm_R": 25,
       "hand_L": 15, "hand_R": 15, "leg_L": 25, "leg_R": 25,
       "foot_L": 15, "foot_R": 15}
ACTD = {"root": 6, "torso": 3, "head": 3, "arm_L": 4, "arm_R": 4,
        "hand_L": 5, "hand_R": 5, "leg_L": 4, "leg_R": 4,
        "foot_L": 3, "foot_R": 3}
RK = ["tracking", "control", "alive"]
B = 32768
H = 256
MIN_STD = 0.1
ENT_W = 0.01
MOTOR_SCALE = 1.0
TOTAL_OBS = 270
TOTAL_ACT = 44

N_CORES = 8
BS = B // N_CORES          # 4096 rows per core
NT = 512                   # batch-tile (free dim per matmul)
NTILES = BS // NT          # 8

LOG2PI = float(np.log(2.0 * np.pi))

OBS_OFF = {}
_o = 0
for _k in MODULES:
    OBS_OFF[_k] = _o
    _o += OBS[_k]

ACT_OFF = {}
_o = 0
for _k in MODULES:
    ACT_OFF[_k] = _o
    _o += ACTD[_k]

# Obs rows are scattered into padded 128-row chunks; every module's rows
# start at a PE-quadrant-legal partition offset (0/32/64).  Padding rows are
# zero in both the obs and the critic L1 weight, so full-128 K-chunks stay
# exact for the critic while per-module slices drive the input layers.
CHUNKS = [(("root", 0), ("torso", 64)),
          (("head", 0), ("arm_L", 32), ("arm_R", 64)),
          (("hand_L", 0), ("hand_R", 32), ("leg_L", 64)),
          (("leg_R", 0), ("foot_L", 32), ("foot_R", 64))]
N_CHUNKS = len(CHUNKS)
PAD_OBS = N_CHUNKS * 128  # 512
MOD_CHUNK = {}
for _ci, _c in enumerate(CHUNKS):
    for _m, _slot in _c:
        assert _slot + OBS[_m] <= 128
        MOD_CHUNK[_m] = (_ci, _slot)

NONROOT = [m for m in MODULES if m != "root"]
MSG_IDX = {}
for _i, _m in enumerate(NONROOT):
    MSG_IDX[("aff", _m)] = _i
    MSG_IDX[("eff", _m)] = 10 + _i

MIDX = {m: i for i, m in enumerate(MODULES)}

MOT_M = 108  # motor psum rows: mean at 0:44, std_p at 64:108 (32-aligned)


def _build_nc(zero_bias: bool):
    nc = bacc.Bacc(None, target_bir_lowering=False)

    # ---- DRAM parameters -------------------------------------------------
    obsT = nc.declare_dram_parameter("obsT", [PAD_OBS, BS], BF16, isOutput=False)
    rawT = nc.declare_dram_parameter("rawT", [TOTAL_ACT, BS], F32, isOutput=False)
    w_inp = nc.declare_dram_parameter("w_inp", [PAD_OBS, H], BF16, isOutput=False)
    w_msg = nc.declare_dram_parameter("w_msg", [128, 20 * 512], BF16, isOutput=False)
    w_mot = nc.declare_dram_parameter("w_mot", [128, 22 * MOT_M], BF16, isOutput=False)
    w_e1 = nc.declare_dram_parameter("w_e1", [PAD_OBS, 512], BF16, isOutput=False)
    w_e2 = nc.declare_dram_parameter("w_e2", [128, 2048], BF16, isOutput=False)
    w_e3 = nc.declare_dram_parameter("w_e3", [128, 1024], BF16, isOutput=False)
    w_hd = nc.declare_dram_parameter("w_hd", [128, 6], BF16, isOutput=False)
    b_mot = nc.declare_dram_parameter("b_mot", [MOT_M, 1], F32, isOutput=False)
    b_hd = nc.declare_dram_parameter("b_hd", [3, 1], F32, isOutput=False)
    if not zero_bias:
        b_inp = nc.declare_dram_parameter("b_inp", [128, 22], F32, isOutput=False)
        b_msg = nc.declare_dram_parameter("b_msg", [128, 40], F32, isOutput=False)
        b_e1 = nc.declare_dram_parameter("b_e1", [128, 4], F32, isOutput=False)
        b_e2 = nc.declare_dram_parameter("b_e2", [128, 4], F32, isOutput=False)
        b_e3 = nc.declare_dram_parameter("b_e3", [128, 2], F32, isOutput=False)

    out_act = nc.declare_dram_parameter("out_act", [TOTAL_ACT, BS], F32, isOutput=True)
    out_ll = nc.declare_dram_parameter("out_ll", [1, BS], F32, isOutput=True)
    out_vals = nc.declare_dram_parameter("out_vals", [3, BS], F32, isOutput=True)
    out_ent = nc.declare_dram_parameter("out_ent", [TOTAL_ACT, 1], F32, isOutput=True)

    with TileContext(nc) as tc:
        with (
            tc.tile_pool(name="spool", bufs=1) as spool,
            tc.tile_pool(name="ppool", bufs=2, space="PSUM") as ppool,
        ):
            # ---- batch-lifetime staging tiles (all fp32) ----------------
            mstd_sb = spool.tile([MOT_M, BS], F32, tag="mstd")
            # values rows 0:3, loglik row 32
            vl_sb = spool.tile([33, BS], F32, tag="vl")
            act_sb = spool.tile([TOTAL_ACT, BS], F32, tag="act")   # tanh(raw)
            d2_sb = spool.tile([TOTAL_ACT, BS], F32, tag="d2")     # (raw-mean)^2
            u_sb = spool.tile([TOTAL_ACT, BS], F32, tag="u")       # tanh^2
            entp_sb = spool.tile([TOTAL_ACT, 1], F32, tag="entp")
            ones_sb = spool.tile([TOTAL_ACT, 1], F32, tag="ones")
            nc.vector.memset(ones_sb[:], 1.0)
            c_minstd = spool.tile([TOTAL_ACT, 1], F32, tag="cmin")
            nc.vector.memset(c_minstd[:], MIN_STD)
            c_eps = spool.tile([TOTAL_ACT, 1], F32, tag="ceps")
            nc.vector.memset(c_eps[:], 1.0 + 1e-6)

            def mm(out, lhsT, rhs, start, stop):
                nc.tensor.matmul(out, lhsT, rhs, start=start, stop=stop)

            # ================= phase A: network + motor ==================
            with (
                tc.tile_pool(name="wpool", bufs=1) as wpool,
                tc.tile_pool(name="obspool", bufs=2) as obspool,
                tc.tile_pool(name="rawpool", bufs=2) as rawpool,
                tc.tile_pool(name="xpool", bufs=2) as xpool,
                tc.tile_pool(name="tpool", bufs=2) as tpool,
                tc.tile_pool(name="rpool", bufs=1) as rpool,
                tc.tile_pool(name="hpool", bufs=1) as hpool,
            ):
                # ---- persistent weight tiles ----------------------------
                w_inp_sb = []
                w_e1_sb = []
                for ci in range(N_CHUNKS):
                    ti = wpool.tile([128, H], BF16, tag=f"winp{ci}")
                    nc.sync.dma_start(out=ti[:], in_=w_inp[ci * 128:(ci + 1) * 128, :])
                    w_inp_sb.append(ti)
                w_msg_sb = wpool.tile([128, 20 * 512], BF16, tag="wmsg")
                for q in range(4):
                    nc.sync.dma_start(out=w_msg_sb[:, q * 2560:(q + 1) * 2560],
                                      in_=w_msg[:, q * 2560:(q + 1) * 2560])
                w_mot_sb = wpool.tile([128, 22 * MOT_M], BF16, tag="wmot")
                nc.sync.dma_start(out=w_mot_sb[:], in_=w_mot[:])
                for ci in range(N_CHUNKS):
                    te = wpool.tile([128, 512], BF16, tag=f"we1{ci}")
                    nc.sync.dma_start(out=te[:], in_=w_e1[ci * 128:(ci + 1) * 128, :])
                    w_e1_sb.append(te)
                w_e2_sb = wpool.tile([128, 2048], BF16, tag="we2")
                nc.sync.dma_start(out=w_e2_sb[:], in_=w_e2[:])
                w_e3_sb = wpool.tile([128, 1024], BF16, tag="we3")
                nc.sync.dma_start(out=w_e3_sb[:], in_=w_e3[:])
                w_hd_sb = wpool.tile([128, 6], BF16, tag="whd")
                nc.sync.dma_start(out=w_hd_sb[:], in_=w_hd[:])
                b_mot_sb = wpool.tile([MOT_M, 1], F32, tag="bmot")
                nc.sync.dma_start(out=b_mot_sb[:], in_=b_mot[:])
                b_hd_sb = wpool.tile([3, 1], F32, tag="bhd")
                nc.sync.dma_start(out=b_hd_sb[:], in_=b_hd[:])
                if not zero_bias:
                    b_inp_sb = wpool.tile([128, 22], F32, tag="binp")
                    nc.sync.dma_start(out=b_inp_sb[:], in_=b_inp[:])
                    b_msg_sb = wpool.tile([128, 40], F32, tag="bmsg")
                    nc.sync.dma_start(out=b_msg_sb[:], in_=b_msg[:])
                    b_e1_sb = wpool.tile([128, 4], F32, tag="be1")
                    nc.sync.dma_start(out=b_e1_sb[:], in_=b_e1[:])
                    b_e2_sb = wpool.tile([128, 4], F32, tag="be2")
                    nc.sync.dma_start(out=b_e2_sb[:], in_=b_e2[:])
                    b_e3_sb = wpool.tile([128, 2], F32, tag="be3")
                    nc.sync.dma_start(out=b_e3_sb[:], in_=b_e3[:])

                for t in range(NTILES):
                    ct = t * NT

                    # xall: module k at cols [k*2*NT, (k+1)*2*NT); within a
                    # module: cols 0:NT = features 0:128, NT:2NT = 128:256
                    xall = xpool.tile([128, 11 * 2 * NT], BF16, tag="xall")

                    def xsl(k, kc=None):
                        base = MIDX[k] * 2 * NT
                        if kc is None:
                            return xall[:, base:base + 2 * NT]
                        return xall[:, base + kc * NT: base + (kc + 1) * NT]

                    def evict_swish(ps_ap, dst_ap, nblk, bias_cols):
                        if zero_bias:
                            nc.scalar.activation(dst_ap, ps_ap, AF.Silu)
                        else:
                            for i in range(nblk):
                                bt, bc = bias_cols[i]
                                nc.scalar.activation(
                                    dst_ap[:, i * NT:(i + 1) * NT],
                                    ps_ap[:, i * NT:(i + 1) * NT],
                                    AF.Silu, bias=bt[:, bc:bc + 1])

                    # -- load obs chunks + raw slice ----------------------
                    obs_t = []
                    for ci in range(N_CHUNKS):
                        ob = obspool.tile([128, NT], BF16, tag=f"obs{ci}")
                        nc.sync.dma_start(
                            out=ob[:],
                            in_=obsT[ci * 128:(ci + 1) * 128, ct:ct + NT])
                        obs_t.append(ob)
                    raw_t = rawpool.tile([TOTAL_ACT, NT], F32, tag="raw")
                    nc.sync.dma_start(out=raw_t[:], in_=rawT[:, ct:ct + NT])
                    # tanh(raw) lives in the silu table set -> in-loop
                    nc.scalar.activation(act_sb[:, ct:ct + NT], raw_t[:], AF.Tanh)
                    nc.vector.tensor_mul(u_sb[:, ct:ct + NT],
                                         act_sb[:, ct:ct + NT],
                                         act_sb[:, ct:ct + NT])

                    # -- input layers: pairs of adjacent modules ----------
                    pairs = [("root", "torso"), ("head", "arm_L"),
                             ("arm_R", "hand_L"), ("hand_R", "leg_L"),
                             ("leg_R", "foot_L"), ("foot_R",)]
                    for pr in pairs:
                        pw = len(pr) * 2 * NT
                        ps = ppool.tile([128, 2048], F32, tag="ps")
                        for j, k in enumerate(pr):
                            ci, ro = MOD_CHUNK[k]
                            d = OBS[k]
                            for mc in range(2):
                                mm(ps[:, j * 2 * NT + mc * NT:
                                      j * 2 * NT + (mc + 1) * NT],
                                   w_inp_sb[ci][ro:ro + d, mc * 128:(mc + 1) * 128],
                                   obs_t[ci][ro:ro + d, :],
                                   start=True, stop=True)
                        base = MIDX[pr[0]] * 2 * NT
                        bias_cols = []
                        if not zero_bias:
                            for k in pr:
                                for mc in range(2):
                                    bias_cols.append((b_inp_sb, MIDX[k] * 2 + mc))
                        evict_swish(ps[:, 0:pw], xall[:, base:base + pw],
                                    len(pr) * 2, bias_cols)

                    # -- message-passing pair: swish(W.T @ src) -> tmp ----
                    def msg_pair(kind, mods, srcs):
                        ps = ppool.tile([128, 2048], F32, tag="ps")
                        for j, (k, src) in enumerate(zip(mods, srcs)):
                            mi = MSG_IDX[(kind, k)]
                            wb = mi * 512
                            for mc in range(2):
                                for kc in range(2):
                                    mm(ps[:, j * 2 * NT + mc * NT:
                                          j * 2 * NT + (mc + 1) * NT],
                                       w_msg_sb[:, wb + kc * 256 + mc * 128:
                                                wb + kc * 256 + (mc + 1) * 128],
                                       src[:, kc * NT:(kc + 1) * NT],
                                       start=(kc == 0), stop=(kc == 1))
                        tmp = tpool.tile([128, 2048], BF16, tag="msgtmp")
                        w = len(mods) * 2 * NT
                        bias_cols = []
                        if not zero_bias:
                            for k in mods:
                                mi = MSG_IDX[(kind, k)]
                                for mc in range(2):
                                    bias_cols.append((b_msg_sb, mi * 2 + mc))
                        evict_swish(ps[:, 0:w], tmp[:, 0:w], len(mods) * 2, bias_cols)
                        return tmp

                    # afferent leaf -> mid (merged adds over adjacent dst)
                    tmp = msg_pair("aff", ["hand_L", "hand_R"],
                                   [xsl("hand_L"), xsl("hand_R")])
                    d0 = MIDX["arm_L"] * 2 * NT
                    nc.vector.tensor_add(xall[:, d0:d0 + 4 * NT],
                                         xall[:, d0:d0 + 4 * NT], tmp[:, 0:4 * NT])
                    tmp = msg_pair("aff", ["foot_L", "foot_R"],
                                   [xsl("foot_L"), xsl("foot_R")])
                    d0 = MIDX["leg_L"] * 2 * NT
                    nc.vector.tensor_add(xall[:, d0:d0 + 4 * NT],
                                         xall[:, d0:d0 + 4 * NT], tmp[:, 0:4 * NT])
                    # afferent mid -> root: 3 pairs, tree-reduced
                    rts = []
                    for pi, mods in enumerate((["arm_L", "arm_R"],
                                               ["leg_L", "leg_R"],
                                               ["torso", "head"])):
                        tmp = msg_pair("aff", mods, [xsl(mods[0]), xsl(mods[1])])
                        rt = rpool.tile([128, 2 * NT], BF16, tag=f"rt{pi}")
                        nc.vector.tensor_add(rt[:], tmp[:, 0:2 * NT],
                                             tmp[:, 2 * NT:4 * NT])
                        rts.append(rt)
                    nc.vector.tensor_add(rts[0][:], rts[0][:], rts[1][:])
                    nc.vector.tensor_add(rts[0][:], rts[0][:], rts[2][:])
                    nc.vector.tensor_add(xsl("root"), xsl("root"), rts[0][:])

                    # efferent root wave
                    for mods in (["torso", "head"], ["arm_L", "arm_R"],
                                 ["leg_L", "leg_R"]):
                        tmp = msg_pair("eff", mods, [xsl("root"), xsl("root")])
                        d0 = MIDX[mods[0]] * 2 * NT
                        nc.vector.tensor_add(xall[:, d0:d0 + 4 * NT],
                                             xall[:, d0:d0 + 4 * NT],
                                             tmp[:, 0:4 * NT])
                    # efferent leaf wave
                    tmp = msg_pair("eff", ["hand_L", "hand_R"],
                                   [xsl("arm_L"), xsl("arm_R")])
                    d0 = MIDX["hand_L"] * 2 * NT
                    nc.vector.tensor_add(xall[:, d0:d0 + 4 * NT],
                                         xall[:, d0:d0 + 4 * NT], tmp[:, 0:4 * NT])
                    tmp = msg_pair("eff", ["foot_L", "foot_R"],
                                   [xsl("leg_L"), xsl("leg_R")])
                    d0 = MIDX["foot_L"] * 2 * NT
                    nc.vector.tensor_add(xall[:, d0:d0 + 4 * NT],
                                         xall[:, d0:d0 + 4 * NT], tmp[:, 0:4 * NT])

                    # -- motor heads: 22 accumulating matmuls -> [108, NT] -
                    psm = ppool.tile([128, 2048], F32, tag="ps")
                    nmm = 0
                    for k in MODULES:
                        for kc in range(2):
                            blk = (2 * MIDX[k] + kc) * MOT_M
                            mm(psm[0:MOT_M, 0:NT], w_mot_sb[:, blk:blk + MOT_M],
                               xsl(k, kc), start=(nmm == 0), stop=(nmm == 21))
                            nmm += 1
                    nc.vector.tensor_scalar_add(mstd_sb[:, ct:ct + NT],
                                                psm[0:MOT_M, 0:NT],
                                                b_mot_sb[0:MOT_M, 0:1])
                    # d2 = (raw - mean)^2, computed in-loop (fp32)
                    nc.vector.tensor_sub(d2_sb[:, ct:ct + NT], raw_t[:],
                                         mstd_sb[0:44, ct:ct + NT])
                    nc.vector.tensor_mul(d2_sb[:, ct:ct + NT],
                                         d2_sb[:, ct:ct + NT],
                                         d2_sb[:, ct:ct + NT])

                    # -- critic encoder -----------------------------------
                    ps1 = ppool.tile([128, 2048], F32, tag="ps")
                    for mc in range(4):
                        for ci in range(N_CHUNKS):
                            mm(ps1[:, mc * NT:(mc + 1) * NT],
                               w_e1_sb[ci][:, mc * 128:(mc + 1) * 128],
                               obs_t[ci][:],
                               start=(ci == 0), stop=(ci == N_CHUNKS - 1))
                    h1 = hpool.tile([128, 2048], BF16, tag="h1")
                    if zero_bias:
                        nc.scalar.activation(h1[:], ps1[:], AF.Silu)
                    else:
                        for mc in range(4):
                            nc.scalar.activation(h1[:, mc * NT:(mc + 1) * NT],
                                                 ps1[:, mc * NT:(mc + 1) * NT],
                                                 AF.Silu, bias=b_e1_sb[:, mc:mc + 1])

                    ps2 = ppool.tile([128, 2048], F32, tag="ps")
                    for mc in range(4):
                        for kc in range(4):
                            mm(ps2[:, mc * NT:(mc + 1) * NT],
                               w_e2_sb[:, kc * 512 + mc * 128:
                                       kc * 512 + (mc + 1) * 128],
                               h1[:, kc * NT:(kc + 1) * NT],
                               start=(kc == 0), stop=(kc == 3))
                    h2 = hpool.tile([128, 2048], BF16, tag="h2")
                    if zero_bias:
                        nc.scalar.activation(h2[:], ps2[:], AF.Silu)
                    else:
                        for mc in range(4):
                            nc.scalar.activation(h2[:, mc * NT:(mc + 1) * NT],
                                                 ps2[:, mc * NT:(mc + 1) * NT],
                                                 AF.Silu, bias=b_e2_sb[:, mc:mc + 1])

                    ps3 = ppool.tile([128, 2048], F32, tag="ps")
                    for mc in range(2):
                        for kc in range(4):
                            mm(ps3[:, mc * NT:(mc + 1) * NT],
                               w_e3_sb[:, kc * 256 + mc * 128:
                                       kc * 256 + (mc + 1) * 128],
                               h2[:, kc * NT:(kc + 1) * NT],
                               start=(kc == 0), stop=(kc == 3))
                    h3 = hpool.tile([128, 1024], BF16, tag="h3")
                    if zero_bias:
                        nc.scalar.activation(h3[:], ps3[:, 0:1024], AF.Silu)
                    else:
                        for mc in range(2):
                            nc.scalar.activation(h3[:, mc * NT:(mc + 1) * NT],
                                                 ps3[:, mc * NT:(mc + 1) * NT],
                                                 AF.Silu, bias=b_e3_sb[:, mc:mc + 1])

                    psv = ppool.tile([128, 2048], F32, tag="ps")
                    for kc in range(2):
                        mm(psv[0:3, 0:NT], w_hd_sb[:, kc * 3:(kc + 1) * 3],
                           h3[:, kc * NT:(kc + 1) * NT],
                           start=(kc == 0), stop=(kc == 1))
                    nc.vector.tensor_scalar_add(vl_sb[0:3, ct:ct + NT],
                                                psv[0:3, 0:NT], b_hd_sb[0:3, 0:1])

            # ================= phase B: distribution math ================
            with tc.tile_pool(name="dpool", bufs=1) as dpool:
                mean = mstd_sb[0:44, :]
                stdp = mstd_sb[64:64 + 44, :]
                bufA = dpool.tile([TOTAL_ACT, BS], F32, tag="bufA")
                bufB = dpool.tile([TOTAL_ACT, BS], F32, tag="bufB")
                bufC = dpool.tile([TOTAL_ACT, BS], F32, tag="bufC")

                nc.scalar.activation(bufA[:], stdp, AF.Exp)                 # e^stdp
                nc.scalar.activation(bufB[:], bufA[:], AF.Ln, bias=1.0)     # softplus
                nc.scalar.activation(bufA[:], bufB[:], AF.Ln, bias=c_minstd[:],
                                     accum_out=entp_sb[:])                  # logstd
                nc.scalar.activation(bufB[:], bufA[:], AF.Exp, scale=-2.0)  # 1/std^2
                nc.vector.tensor_mul(bufC[:], d2_sb[:], bufB[:])            # z^2
                # B = -0.5*z^2 - logstd
                nc.vector.scalar_tensor_tensor(bufB[:], bufC[:], -0.5, bufA[:],
                                               op0=ALU.mult, op1=ALU.subtract)
                nc.scalar.activation(bufA[:], u_sb[:], AF.Ln,
                                     scale=-1.0, bias=c_eps[:])             # tterm
                nc.vector.tensor_sub(bufC[:], bufB[:], bufA[:])             # lp

                # loglik = colsum(lp) - 22*log(2pi), exact fp32 ones-matmul
                for t in range(NTILES):
                    ct = t * NT
                    pl = ppool.tile([128, 2048], F32, tag="ps")
                    nc.tensor.matmul(pl[0:1, 0:NT], ones_sb[:], bufC[:, ct:ct + NT],
                                     start=True, stop=True)
                    nc.vector.tensor_scalar_add(vl_sb[32:33, ct:ct + NT],
                                                pl[0:1, 0:NT],
                                                float(-22.0 * LOG2PI))

                # ---- output DMAs ----------------------------------------
                nc.sync.dma_start(out=out_act[:], in_=act_sb[:])
                nc.sync.dma_start(out=out_ll[:], in_=vl_sb[32:33, :])
                nc.sync.dma_start(out=out_vals[:], in_=vl_sb[0:3, :])
                nc.sync.dma_start(out=out_ent[:], in_=entp_sb[:])

    nc.finalize()
    return nc


# ---------------------------------------------------------------------------
# Host-side weight packing
# ---------------------------------------------------------------------------

def _pack_msg(Ws):
    """20 [256,256] matrices -> [128, 20*512]; (kc,mc) block at j*512+kc*256+mc*128."""
    out = np.zeros((128, len(Ws) * 512), np.float32)
    for j, W in enumerate(Ws):
        blk = W.reshape(2, 128, 2, 128).transpose(1, 0, 2, 3).reshape(128, 512)
        out[:, j * 512:(j + 1) * 512] = blk
    return out.astype(BF)


def _pack_e2(W):  # [512, 512] -> [128, 4*512]
    return np.ascontiguousarray(
        W.reshape(4, 128, 512).transpose(1, 0, 2).reshape(128, 2048)).astype(BF)


def _pack_e3(W):  # [512, 256] -> [128, 4*256]
    return np.ascontiguousarray(
        W.reshape(4, 128, 256).transpose(1, 0, 2).reshape(128, 1024)).astype(BF)


def _pack_hd(W):  # [256, 3] -> [128, 2*3]
    out = np.zeros((128, 6), np.float32)
    for kc in range(2):
        out[:, kc * 3:(kc + 1) * 3] = W[kc * 128:(kc + 1) * 128, :]
    return out.astype(BF)


def _pack_mot(params):
    """Block-diagonal motor weights: [128, 22*108].

    K-chunk (module k, kc) at cols (2*MIDX[k]+kc)*108; within the 108 columns,
    module k's mean cols land at ACT_OFF[k].. and std cols at 64+ACT_OFF[k]..
    (row-64 start keeps the std block partition-base 32-aligned on chip).
    """
    out = np.zeros((128, 22 * MOT_M), np.float32)
    for k in MODULES:
        W = np.asarray(params["motor"][k][0], np.float32) * MOTOR_SCALE
        a = ACTD[k]
        for kc in range(2):
            blk = W[kc * 128:(kc + 1) * 128, :]
            base = (2 * MIDX[k] + kc) * MOT_M
            out[:, base + ACT_OFF[k]: base + ACT_OFF[k] + a] = blk[:, :a]
            out[:, base + 64 + ACT_OFF[k]: base + 64 + ACT_OFF[k] + a] = blk[:, a:]
    return out.astype(BF)


_NC_CACHE = {}
LAST_RESULT = None


def kernel(obs, raw_action, params):
    global LAST_RESULT
    obs = {k: np.asarray(v, np.float32) for k, v in obs.items()}
    raw = {k: np.asarray(v, np.float32) for k, v in raw_action.items()}

    def P(x):
        return np.asarray(x, np.float32)

    zero_bias = all([
        all(not P(params["inp"][k][1]).any() for k in MODULES),
        all(not P(params["aff"][k][1]).any() for k in NONROOT),
        all(not P(params["eff"][k][1]).any() for k in NONROOT),
        all(not P(p[1]).any() for p in params["enc"]),
    ])

    if zero_bias not in _NC_CACHE:
        _NC_CACHE[zero_bias] = _build_nc(zero_bias)
    nc = _NC_CACHE[zero_bias]

    # ---- pack host arrays ------------------------------------------------
    obsT = np.zeros((PAD_OBS, B), BF)
    for k in MODULES:
        ci, slot = MOD_CHUNK[k]
        obsT[ci * 128 + slot: ci * 128 + slot + OBS[k], :] = obs[k].T.astype(BF)
    rawT = np.ascontiguousarray(
        np.concatenate([raw[k] for k in MODULES], axis=1).T)        # [44, B]

    w_inp = np.zeros((PAD_OBS, H), np.float32)
    for k in MODULES:
        ci, slot = MOD_CHUNK[k]
        w_inp[ci * 128 + slot: ci * 128 + slot + OBS[k], :] = P(params["inp"][k][0])
    w_inp = w_inp.astype(BF)
    msg_mats = [P(params["aff"][k][0]) for k in NONROOT] + \
               [P(params["eff"][k][0]) for k in NONROOT]
    w_msg = _pack_msg(msg_mats)
    w_mot = _pack_mot(params)
    w_e1_orig = P(params["enc"][0][0])                               # [270, 512]
    w_e1 = np.zeros((PAD_OBS, 512), np.float32)
    for k in MODULES:
        ci, slot = MOD_CHUNK[k]
        w_e1[ci * 128 + slot: ci * 128 + slot + OBS[k], :] = \
            w_e1_orig[OBS_OFF[k]:OBS_OFF[k] + OBS[k], :]
    w_e1 = w_e1.astype(BF)
    w_e2 = _pack_e2(P(params["enc"][1][0]))
    w_e3 = _pack_e3(P(params["enc"][2][0]))
    w_hd = _pack_hd(np.concatenate([P(params["heads"][r][0]) for r in RK], axis=1))

    b_mot = np.zeros((MOT_M, 1), np.float32)
    for k in MODULES:
        bv = P(params["motor"][k][1]) * MOTOR_SCALE
        a = ACTD[k]
        b_mot[ACT_OFF[k]:ACT_OFF[k] + a, 0] = bv[:a]
        b_mot[64 + ACT_OFF[k]:64 + ACT_OFF[k] + a, 0] = bv[a:]
    b_hd = np.stack([P(params["heads"][r][1]) for r in RK]).reshape(3, 1)

    shared = {
        "w_inp": w_inp, "w_msg": w_msg, "w_mot": w_mot, "w_e1": w_e1,
        "w_e2": w_e2, "w_e3": w_e3, "w_hd": w_hd, "b_mot": b_mot, "b_hd": b_hd,
    }
    if not zero_bias:
        b_inp = np.zeros((128, 22), np.float32)
        for k in MODULES:
            bv = P(params["inp"][k][1])
            for mc in range(2):
                b_inp[:, MIDX[k] * 2 + mc] = bv[mc * 128:(mc + 1) * 128]
        b_msg = np.zeros((128, 40), np.float32)
        for kind in ("aff", "eff"):
            for k in NONROOT:
                mi = MSG_IDX[(kind, k)]
                bv = P(params[kind][k][1])
                for mc in range(2):
                    b_msg[:, mi * 2 + mc] = bv[mc * 128:(mc + 1) * 128]
        b_e1 = P(params["enc"][0][1]).reshape(4, 128).T.copy()
        b_e2 = P(params["enc"][1][1]).reshape(4, 128).T.copy()
        b_e3 = P(params["enc"][2][1]).reshape(2, 128).T.copy()
        shared.update({"b_inp": b_inp, "b_msg": b_msg, "b_e1": b_e1,
                       "b_e2": b_e2, "b_e3": b_e3})

    in_maps = []
    for i in range(N_CORES):
        m = dict(shared)
        m["obsT"] = np.ascontiguousarray(obsT[:, i * BS:(i + 1) * BS])
        m["rawT"] = np.ascontiguousarray(rawT[:, i * BS:(i + 1) * BS])
        in_maps.append(m)

    trace = bool(int(os.environ.get("KERNEL_TRACE", "0")))
    res = run_bass_kernel_spmd(nc, in_maps, list(range(N_CORES)), trace=trace)
    LAST_RESULT = res

    out = np.empty((B, 49), np.float32)
    ent_sum = 0.0
    for i in range(N_CORES):
        r = res.results[i]
        sl = slice(i * BS, (i + 1) * BS)
        out[sl, 0:44] = r["out_act"].T
        out[sl, 44] = r["out_ll"][0]
        out[sl, 45:48] = r["out_vals"].T
        ent_sum += float(r["out_ent"].sum())
    ent = 22.0 * (1.0 + LOG2PI) + ent_sum / B
    out[:, 48] = -ENT_W * ent
    return out


# revision 21
# speedup vs baseline: 1.1825x; 1.0041x over previous
"""NerveNet MLP critic network — Trainium2 Bass kernel (8-core data parallel).

Layout strategy: everything runs feature-major (features on SBUF partitions,
batch on the free axis), so every GEMM is `out = W.T @ xT` with the weight as
the PE stationary operand and the transposed activations streaming.  Inputs
are transposed on the host; the output is transposed back on the host.

GEMM operands are bf16 (weight loads hide under FWL, matmuls issue
back-to-back at N cycles); all accumulation is fp32 in PSUM, and the entire
tanh-Normal log-likelihood / entropy path runs in fp32.  Swish eviction
PSUM->SBUF is fused into ScalarE activation instructions spanning 4 PSUM
banks.  The ln/exp distribution math is deferred to a single post-loop phase
so the ScalarE activation-table set only switches once.
"""

import os
import sys
import types
import math

sys.path.insert(0, "/opt/trn_rl_repo")

import numpy as np
import ml_dtypes

BF = ml_dtypes.bfloat16

# ---------------------------------------------------------------------------
# antenv.axon_hooks shim (enables NTFF profiling under axon in this image)
# ---------------------------------------------------------------------------
if "antenv.axon_hooks" not in sys.modules:
    try:
        import antenv  # noqa: F401

        _mod = types.ModuleType("antenv.axon_hooks")
        _hook_box = [None]
        _mod.set_axon_ntff_profile_hook = lambda h: _hook_box.__setitem__(0, h)
        _mod.get_axon_ntff_profile_hook = lambda: _hook_box[0]
        sys.modules["antenv.axon_hooks"] = _mod
        from trn_agent_boot.trn_boot import _ntff_profile_via_ctypes

        _mod.set_axon_ntff_profile_hook(
            _ntff_profile_via_ctypes("/opt/axon/libaxon_pjrt.so")
        )
    except Exception:
        pass

import concourse.bacc as bacc
import concourse.mybir as mybir
from concourse.tile import TileContext
from concourse.bass_utils import run_bass_kernel_spmd

F32 = mybir.dt.float32
BF16 = mybir.dt.bfloat16
AF = mybir.ActivationFunctionType
ALU = mybir.AluOpType

# ---------------------------------------------------------------------------
# Problem constants (hardcoded per the task contract)
# ---------------------------------------------------------------------------
MODULES = ["root", "torso", "head", "arm_L", "arm_R", "hand_L", "hand_R",
           "leg_L", "leg_R", "foot_L", "foot_R"]
OBS = {"root": 60, "torso": 30, "head": 20, "arm_L": 25, "arm_R": 25,
       "hand_L": 15, "hand_R": 15, "leg_L": 25, "leg_R": 25,
       "foot_L": 15, "foot_R": 15}
ACTD = {"root": 6, "torso": 3, "head": 3, "arm_L": 4, "arm_R": 4,
        "hand_L": 5, "hand_R": 5, "leg_L": 4, "leg_R": 4,
        "foot_L": 3, "foot_R": 3}
RK = ["tracking", "control", "alive"]
B = 32768
H = 256
MIN_STD = 0.1
ENT_W = 0.01
MOTOR_SCALE = 1.0
TOTAL_OBS = 270
TOTAL_ACT = 44

N_CORES = 8
BS = B // N_CORES          # 4096 rows per core
NT = 512                   # batch-tile (free dim per matmul)
NTILES = BS // NT          # 8

LOG2PI = float(np.log(2.0 * np.pi))

OBS_OFF = {}
_o = 0
for _k in MODULES:
    OBS_OFF[_k] = _o
    _o += OBS[_k]

ACT_OFF = {}
_o = 0
for _k in MODULES:
    ACT_OFF[_k] = _o
    _o += ACTD[_k]

# Obs rows are scattered into padded 128-row chunks; every module's rows
# start at a PE-quadrant-legal partition offset (0/32/64).  Padding rows are
# zero in both the obs and the critic L1 weight, so full-128 K-chunks stay
# exact for the critic while per-module slices drive the input layers.
CHUNKS = [(("root", 0), ("torso", 64)),
          (("head", 0), ("arm_L", 32), ("arm_R", 64)),
          (("hand_L", 0), ("hand_R", 32), ("leg_L", 64)),
          (("leg_R", 0), ("foot_L", 32), ("foot_R", 64))]
N_CHUNKS = len(CHUNKS)
PAD_OBS = N_CHUNKS * 128  # 512
MOD_CHUNK = {}
for _ci, _c in enumerate(CHUNKS):
    for _m, _slot in _c:
        assert _slot + OBS[_m] <= 128
        MOD_CHUNK[_m] = (_ci, _slot)

NONROOT = [m for m in MODULES if m != "root"]
MSG_IDX = {}
for _i, _m in enumerate(NONROOT):
    MSG_IDX[("aff", _m)] = _i
    MSG_IDX[("eff", _m)] = 10 + _i

MIDX = {m: i for i, m in enumerate(MODULES)}

MOT_M = 108  # motor psum rows: mean at 0:44, std_p at 64:108 (32-aligned)


def _build_nc(zero_bias: bool):
    nc = bacc.Bacc(None, target_bir_lowering=False)

    # ---- DRAM parameters -------------------------------------------------
    obsT = nc.declare_dram_parameter("obsT", [PAD_OBS, BS], BF16, isOutput=False)
    rawT = nc.declare_dram_parameter("rawT", [TOTAL_ACT, BS], F32, isOutput=False)
    w_inp = nc.declare_dram_parameter("w_inp", [PAD_OBS, H], BF16, isOutput=False)
    w_msg = nc.declare_dram_parameter("w_msg", [128, 20 * 512], BF16, isOutput=False)
    w_mot = nc.declare_dram_parameter("w_mot", [128, 22 * MOT_M], BF16, isOutput=False)
    w_e1 = nc.declare_dram_parameter("w_e1", [PAD_OBS, 512], BF16, isOutput=False)
    w_e2 = nc.declare_dram_parameter("w_e2", [128, 2048], BF16, isOutput=False)
    w_e3 = nc.declare_dram_parameter("w_e3", [128, 1024], BF16, isOutput=False)
    w_hd = nc.declare_dram_parameter("w_hd", [128, 6], BF16, isOutput=False)
    b_mot = nc.declare_dram_parameter("b_mot", [MOT_M, 1], F32, isOutput=False)
    b_hd = nc.declare_dram_parameter("b_hd", [3, 1], F32, isOutput=False)
    if not zero_bias:
        b_inp = nc.declare_dram_parameter("b_inp", [128, 22], F32, isOutput=False)
        b_msg = nc.declare_dram_parameter("b_msg", [128, 40], F32, isOutput=False)
        b_e1 = nc.declare_dram_parameter("b_e1", [128, 4], F32, isOutput=False)
        b_e2 = nc.declare_dram_parameter("b_e2", [128, 4], F32, isOutput=False)
        b_e3 = nc.declare_dram_parameter("b_e3", [128, 2], F32, isOutput=False)

    out_act = nc.declare_dram_parameter("out_act", [TOTAL_ACT, BS], F32, isOutput=True)
    out_ll = nc.declare_dram_parameter("out_ll", [1, BS], F32, isOutput=True)
    out_vals = nc.declare_dram_parameter("out_vals", [3, BS], F32, isOutput=True)
    out_ent = nc.declare_dram_parameter("out_ent", [TOTAL_ACT, 1], F32, isOutput=True)

    with TileContext(nc) as tc:
        with (
            tc.tile_pool(name="spool", bufs=1) as spool,
            tc.tile_pool(name="ppool", bufs=2, space="PSUM") as ppool,
        ):
            # ---- batch-lifetime staging tiles (all fp32) ----------------
            mstd_sb = spool.tile([MOT_M, BS], F32, tag="mstd")
            # values rows 0:3, loglik row 32
            vl_sb = spool.tile([33, BS], F32, tag="vl")
            act_sb = spool.tile([TOTAL_ACT, BS], F32, tag="act")   # tanh(raw)
            d2_sb = spool.tile([TOTAL_ACT, BS], F32, tag="d2")     # (raw-mean)^2
            u_sb = spool.tile([TOTAL_ACT, BS], F32, tag="u")       # tanh^2
            entp_sb = spool.tile([TOTAL_ACT, 1], F32, tag="entp")
            ones_sb = spool.tile([TOTAL_ACT, 1], F32, tag="ones")
            nc.vector.memset(ones_sb[:], 1.0)
            c_minstd = spool.tile([TOTAL_ACT, 1], F32, tag="cmin")
            nc.vector.memset(c_minstd[:], MIN_STD)
            c_eps = spool.tile([TOTAL_ACT, 1], F32, tag="ceps")
            nc.vector.memset(c_eps[:], 1.0 + 1e-6)

            def mm(out, lhsT, rhs, start, stop):
                nc.tensor.matmul(out, lhsT, rhs, start=start, stop=stop)

            # ================= phase A: network + motor ==================
            with (
                tc.tile_pool(name="wpool", bufs=1) as wpool,
                tc.tile_pool(name="obspool", bufs=2) as obspool,
                tc.tile_pool(name="rawpool", bufs=2) as rawpool,
                tc.tile_pool(name="xpool", bufs=2) as xpool,
                tc.tile_pool(name="tpool", bufs=2) as tpool,
                tc.tile_pool(name="rpool", bufs=1) as rpool,
                tc.tile_pool(name="hpool", bufs=1) as hpool,
            ):
                # ---- persistent weight tiles ----------------------------
                w_inp_sb = []
                w_e1_sb = []
                for ci in range(N_CHUNKS):
                    ti = wpool.tile([128, H], BF16, tag=f"winp{ci}")
                    nc.sync.dma_start(out=ti[:], in_=w_inp[ci * 128:(ci + 1) * 128, :])
                    w_inp_sb.append(ti)
                w_msg_sb = wpool.tile([128, 20 * 512], BF16, tag="wmsg")
                for q in range(4):
                    nc.sync.dma_start(out=w_msg_sb[:, q * 2560:(q + 1) * 2560],
                                      in_=w_msg[:, q * 2560:(q + 1) * 2560])
                w_mot_sb = wpool.tile([128, 22 * MOT_M], BF16, tag="wmot")
                nc.sync.dma_start(out=w_mot_sb[:], in_=w_mot[:])
                for ci in range(N_CHUNKS):
                    te = wpool.tile([128, 512], BF16, tag=f"we1{ci}")
                    nc.sync.dma_start(out=te[:], in_=w_e1[ci * 128:(ci + 1) * 128, :])
                    w_e1_sb.append(te)
                w_e2_sb = wpool.tile([128, 2048], BF16, tag="we2")
                nc.sync.dma_start(out=w_e2_sb[:], in_=w_e2[:])
                w_e3_sb = wpool.tile([128, 1024], BF16, tag="we3")
                nc.sync.dma_start(out=w_e3_sb[:], in_=w_e3[:])
                w_hd_sb = wpool.tile([128, 6], BF16, tag="whd")
                nc.sync.dma_start(out=w_hd_sb[:], in_=w_hd[:])
                b_mot_sb = wpool.tile([MOT_M, 1], F32, tag="bmot")
                nc.sync.dma_start(out=b_mot_sb[:], in_=b_mot[:])
                b_hd_sb = wpool.tile([3, 1], F32, tag="bhd")
                nc.sync.dma_start(out=b_hd_sb[:], in_=b_hd[:])
                if not zero_bias:
                    b_inp_sb = wpool.tile([128, 22], F32, tag="binp")
                    nc.sync.dma_start(out=b_inp_sb[:], in_=b_inp[:])
                    b_msg_sb = wpool.tile([128, 40], F32, tag="bmsg")
                    nc.sync.dma_start(out=b_msg_sb[:], in_=b_msg[:])
                    b_e1_sb = wpool.tile([128, 4], F32, tag="be1")
                    nc.sync.dma_start(out=b_e1_sb[:], in_=b_e1[:])
                    b_e2_sb = wpool.tile([128, 4], F32, tag="be2")
                    nc.sync.dma_start(out=b_e2_sb[:], in_=b_e2[:])
                    b_e3_sb = wpool.tile([128, 2], F32, tag="be3")
                    nc.sync.dma_start(out=b_e3_sb[:], in_=b_e3[:])

                for t in range(NTILES):
                    ct = t * NT

                    # xall: module k at cols [k*2*NT, (k+1)*2*NT); within a
                    # module: cols 0:NT = features 0:128, NT:2NT = 128:256
                    xall = xpool.tile([128, 11 * 2 * NT], BF16, tag="xall")

                    def xsl(k, kc=None):
                        base = MIDX[k] * 2 * NT
                        if kc is None:
                            return xall[:, base:base + 2 * NT]
                        return xall[:, base + kc * NT: base + (kc + 1) * NT]

                    def evict_swish(ps_ap, dst_ap, nblk, bias_cols):
                        if zero_bias:
                            nc.scalar.activation(dst_ap, ps_ap, AF.Silu)
                        else:
                            for i in range(nblk):
                                bt, bc = bias_cols[i]
                                nc.scalar.activation(
                                    dst_ap[:, i * NT:(i + 1) * NT],
                                    ps_ap[:, i * NT:(i + 1) * NT],
                                    AF.Silu, bias=bt[:, bc:bc + 1])

                    # -- load obs chunks + raw slice ----------------------
                    obs_t = []
                    for ci in range(N_CHUNKS):
                        ob = obspool.tile([128, NT], BF16, tag=f"obs{ci}")
                        nc.sync.dma_start(
                            out=ob[:],
                            in_=obsT[ci * 128:(ci + 1) * 128, ct:ct + NT])
                        obs_t.append(ob)
                    raw_t = rawpool.tile([TOTAL_ACT, NT], F32, tag="raw")
                    nc.sync.dma_start(out=raw_t[:], in_=rawT[:, ct:ct + NT])
                    # tanh(raw) lives in the silu table set -> in-loop
                    nc.scalar.activation(act_sb[:, ct:ct + NT], raw_t[:], AF.Tanh)
                    nc.vector.tensor_mul(u_sb[:, ct:ct + NT],
                                         act_sb[:, ct:ct + NT],
                                         act_sb[:, ct:ct + NT])

                    # -- input layers: pairs of adjacent modules ----------
                    pairs = [("root", "torso"), ("head", "arm_L"),
                             ("arm_R", "hand_L"), ("hand_R", "leg_L"),
                             ("leg_R", "foot_L"), ("foot_R",)]
                    for pr in pairs:
                        pw = len(pr) * 2 * NT
                        ps = ppool.tile([128, 2048], F32, tag="ps")
                        for j, k in enumerate(pr):
                            ci, ro = MOD_CHUNK[k]
                            d = OBS[k]
                            for mc in range(2):
                                mm(ps[:, j * 2 * NT + mc * NT:
                                      j * 2 * NT + (mc + 1) * NT],
                                   w_inp_sb[ci][ro:ro + d, mc * 128:(mc + 1) * 128],
                                   obs_t[ci][ro:ro + d, :],
                                   start=True, stop=True)
                        base = MIDX[pr[0]] * 2 * NT
                        bias_cols = []
                        if not zero_bias:
                            for k in pr:
                                for mc in range(2):
                                    bias_cols.append((b_inp_sb, MIDX[k] * 2 + mc))
                        evict_swish(ps[:, 0:pw], xall[:, base:base + pw],
                                    len(pr) * 2, bias_cols)

                    # -- message-passing pair: swish(W.T @ src) -> tmp ----
                    def msg_pair(kind, mods, srcs):
                        ps = ppool.tile([128, 2048], F32, tag="ps")
                        for j, (k, src) in enumerate(zip(mods, srcs)):
                            mi = MSG_IDX[(kind, k)]
                            wb = mi * 512
                            for mc in range(2):
                                for kc in range(2):
                                    mm(ps[:, j * 2 * NT + mc * NT:
                                          j * 2 * NT + (mc + 1) * NT],
                                       w_msg_sb[:, wb + kc * 256 + mc * 128:
                                                wb + kc * 256 + (mc + 1) * 128],
                                       src[:, kc * NT:(kc + 1) * NT],
                                       start=(kc == 0), stop=(kc == 1))
                        tmp = tpool.tile([128, 2048], BF16, tag="msgtmp")
                        w = len(mods) * 2 * NT
                        bias_cols = []
                        if not zero_bias:
                            for k in mods:
                                mi = MSG_IDX[(kind, k)]
                                for mc in range(2):
                                    bias_cols.append((b_msg_sb, mi * 2 + mc))
                        evict_swish(ps[:, 0:w], tmp[:, 0:w], len(mods) * 2, bias_cols)
                        return tmp

                    # -- critic stages as closures, interleaved below -----
                    crit = {}

                    def enc1():
                        ps1 = ppool.tile([128, 2048], F32, tag="ps")
                        for mc in range(4):
                            for ci in range(N_CHUNKS):
                                mm(ps1[:, mc * NT:(mc + 1) * NT],
                                   w_e1_sb[ci][:, mc * 128:(mc + 1) * 128],
                                   obs_t[ci][:],
                                   start=(ci == 0), stop=(ci == N_CHUNKS - 1))
                        h1 = hpool.tile([128, 2048], BF16, tag="h1")
                        if zero_bias:
                            nc.scalar.activation(h1[:], ps1[:], AF.Silu)
                        else:
                            for mc in range(4):
                                nc.scalar.activation(h1[:, mc * NT:(mc + 1) * NT],
                                                     ps1[:, mc * NT:(mc + 1) * NT],
                                                     AF.Silu,
                                                     bias=b_e1_sb[:, mc:mc + 1])
                        crit["h1"] = h1

                    def enc2():
                        h1 = crit["h1"]
                        ps2 = ppool.tile([128, 2048], F32, tag="ps")
                        for mc in range(4):
                            for kc in range(4):
                                mm(ps2[:, mc * NT:(mc + 1) * NT],
                                   w_e2_sb[:, kc * 512 + mc * 128:
                                           kc * 512 + (mc + 1) * 128],
                                   h1[:, kc * NT:(kc + 1) * NT],
                                   start=(kc == 0), stop=(kc == 3))
                        h2 = hpool.tile([128, 2048], BF16, tag="h2")
                        if zero_bias:
                            nc.scalar.activation(h2[:], ps2[:], AF.Silu)
                        else:
                            for mc in range(4):
                                nc.scalar.activation(h2[:, mc * NT:(mc + 1) * NT],
                                                     ps2[:, mc * NT:(mc + 1) * NT],
                                                     AF.Silu,
                                                     bias=b_e2_sb[:, mc:mc + 1])
                        crit["h2"] = h2

                    def enc3():
                        h2 = crit["h2"]
                        ps3 = ppool.tile([128, 2048], F32, tag="ps")
                        for mc in range(2):
                            for kc in range(4):
                                mm(ps3[:, mc * NT:(mc + 1) * NT],
                                   w_e3_sb[:, kc * 256 + mc * 128:
                                           kc * 256 + (mc + 1) * 128],
                                   h2[:, kc * NT:(kc + 1) * NT],
                                   start=(kc == 0), stop=(kc == 3))
                        h3 = hpool.tile([128, 1024], BF16, tag="h3")
                        if zero_bias:
                            nc.scalar.activation(h3[:], ps3[:, 0:1024], AF.Silu)
                        else:
                            for mc in range(2):
                                nc.scalar.activation(h3[:, mc * NT:(mc + 1) * NT],
                                                     ps3[:, mc * NT:(mc + 1) * NT],
                                                     AF.Silu,
                                                     bias=b_e3_sb[:, mc:mc + 1])
                        crit["h3"] = h3

                    def heads():
                        h3 = crit["h3"]
                        psv = ppool.tile([128, 2048], F32, tag="ps")
                        for kc in range(2):
                            mm(psv[0:3, 0:NT], w_hd_sb[:, kc * 3:(kc + 1) * 3],
                               h3[:, kc * NT:(kc + 1) * NT],
                               start=(kc == 0), stop=(kc == 1))
                        nc.vector.tensor_scalar_add(vl_sb[0:3, ct:ct + NT],
                                                    psv[0:3, 0:NT],
                                                    b_hd_sb[0:3, 0:1])

                    enc1()

                    # afferent leaf -> mid (merged adds over adjacent dst)
                    tmp = msg_pair("aff", ["hand_L", "hand_R"],
                                   [xsl("hand_L"), xsl("hand_R")])
                    d0 = MIDX["arm_L"] * 2 * NT
                    nc.vector.tensor_add(xall[:, d0:d0 + 4 * NT],
                                         xall[:, d0:d0 + 4 * NT], tmp[:, 0:4 * NT])
                    tmp = msg_pair("aff", ["foot_L", "foot_R"],
                                   [xsl("foot_L"), xsl("foot_R")])
                    d0 = MIDX["leg_L"] * 2 * NT
                    nc.vector.tensor_add(xall[:, d0:d0 + 4 * NT],
                                         xall[:, d0:d0 + 4 * NT], tmp[:, 0:4 * NT])
                    # afferent mid -> root: 3 pairs, tree-reduced
                    rts = []
                    for pi, mods in enumerate((["arm_L", "arm_R"],
                                               ["leg_L", "leg_R"],
                                               ["torso", "head"])):
                        tmp = msg_pair("aff", mods, [xsl(mods[0]), xsl(mods[1])])
                        rt = rpool.tile([128, 2 * NT], BF16, tag=f"rt{pi}")
                        nc.vector.tensor_add(rt[:], tmp[:, 0:2 * NT],
                                             tmp[:, 2 * NT:4 * NT])
                        rts.append(rt)
                    nc.vector.tensor_add(rts[0][:], rts[0][:], rts[1][:])
                    nc.vector.tensor_add(rts[0][:], rts[0][:], rts[2][:])
                    nc.vector.tensor_add(xsl("root"), xsl("root"), rts[0][:])

                    enc2()

                    # efferent root wave
                    for mods in (["torso", "head"], ["arm_L", "arm_R"],
                                 ["leg_L", "leg_R"]):
                        tmp = msg_pair("eff", mods, [xsl("root"), xsl("root")])
                        d0 = MIDX[mods[0]] * 2 * NT
                        nc.vector.tensor_add(xall[:, d0:d0 + 4 * NT],
                                             xall[:, d0:d0 + 4 * NT],
                                             tmp[:, 0:4 * NT])
                    enc3()

                    # efferent leaf wave
                    tmp = msg_pair("eff", ["hand_L", "hand_R"],
                                   [xsl("arm_L"), xsl("arm_R")])
                    d0 = MIDX["hand_L"] * 2 * NT
                    nc.vector.tensor_add(xall[:, d0:d0 + 4 * NT],
                                         xall[:, d0:d0 + 4 * NT], tmp[:, 0:4 * NT])
                    tmp = msg_pair("eff", ["foot_L", "foot_R"],
                                   [xsl("leg_L"), xsl("leg_R")])
                    d0 = MIDX["foot_L"] * 2 * NT
                    nc.vector.tensor_add(xall[:, d0:d0 + 4 * NT],
                                         xall[:, d0:d0 + 4 * NT], tmp[:, 0:4 * NT])

                    # -- motor heads: 22 accumulating matmuls -> [108, NT] -
                    psm = ppool.tile([128, 2048], F32, tag="ps")
                    nmm = 0
                    for k in MODULES:
                        for kc in range(2):
                            blk = (2 * MIDX[k] + kc) * MOT_M
                            mm(psm[0:MOT_M, 0:NT], w_mot_sb[:, blk:blk + MOT_M],
                               xsl(k, kc), start=(nmm == 0), stop=(nmm == 21))
                            nmm += 1
                    nc.vector.tensor_scalar_add(mstd_sb[:, ct:ct + NT],
                                                psm[0:MOT_M, 0:NT],
                                                b_mot_sb[0:MOT_M, 0:1])
                    # d2 = (raw - mean)^2, computed in-loop (fp32)
                    nc.vector.tensor_sub(d2_sb[:, ct:ct + NT], raw_t[:],
                                         mstd_sb[0:44, ct:ct + NT])
                    nc.vector.tensor_mul(d2_sb[:, ct:ct + NT],
                                         d2_sb[:, ct:ct + NT],
                                         d2_sb[:, ct:ct + NT])

                    heads()

            # ================= phase B: distribution math ================
            with tc.tile_pool(name="dpool", bufs=1) as dpool:
                mean = mstd_sb[0:44, :]
                stdp = mstd_sb[64:64 + 44, :]
                bufA = dpool.tile([TOTAL_ACT, BS], F32, tag="bufA")
                bufB = dpool.tile([TOTAL_ACT, BS], F32, tag="bufB")
                bufC = dpool.tile([TOTAL_ACT, BS], F32, tag="bufC")

                nc.scalar.activation(bufA[:], stdp, AF.Exp)                 # e^stdp
                nc.scalar.activation(bufB[:], bufA[:], AF.Ln, bias=1.0)     # softplus
                nc.scalar.activation(bufA[:], bufB[:], AF.Ln, bias=c_minstd[:],
                                     accum_out=entp_sb[:])                  # logstd
                nc.scalar.activation(bufB[:], bufA[:], AF.Exp, scale=-2.0)  # 1/std^2
                nc.vector.tensor_mul(bufC[:], d2_sb[:], bufB[:])            # z^2
                # B = -0.5*z^2 - logstd
                nc.vector.scalar_tensor_tensor(bufB[:], bufC[:], -0.5, bufA[:],
                                               op0=ALU.mult, op1=ALU.subtract)
                nc.scalar.activation(bufA[:], u_sb[:], AF.Ln,
                                     scale=-1.0, bias=c_eps[:])             # tterm
                nc.vector.tensor_sub(bufC[:], bufB[:], bufA[:])             # lp

                # loglik = colsum(lp) - 22*log(2pi), exact fp32 ones-matmul
                for t in range(NTILES):
                    ct = t * NT
                    pl = ppool.tile([128, 2048], F32, tag="ps")
                    nc.tensor.matmul(pl[0:1, 0:NT], ones_sb[:], bufC[:, ct:ct + NT],
                                     start=True, stop=True)
                    nc.vector.tensor_scalar_add(vl_sb[32:33, ct:ct + NT],
                                                pl[0:1, 0:NT],
                                                float(-22.0 * LOG2PI))

                # ---- output DMAs ----------------------------------------
                nc.sync.dma_start(out=out_act[:], in_=act_sb[:])
                nc.sync.dma_start(out=out_ll[:], in_=vl_sb[32:33, :])
                nc.sync.dma_start(out=out_vals[:], in_=vl_sb[0:3, :])
                nc.sync.dma_start(out=out_ent[:], in_=entp_sb[:])

    nc.finalize()
    return nc


# ---------------------------------------------------------------------------
# Host-side weight packing
# ---------------------------------------------------------------------------

def _pack_msg(Ws):
    """20 [256,256] matrices -> [128, 20*512]; (kc,mc) block at j*512+kc*256+mc*128."""
    out = np.zeros((128, len(Ws) * 512), np.float32)
    for j, W in enumerate(Ws):
        blk = W.reshape(2, 128, 2, 128).transpose(1, 0, 2, 3).reshape(128, 512)
        out[:, j * 512:(j + 1) * 512] = blk
    return out.astype(BF)


def _pack_e2(W):  # [512, 512] -> [128, 4*512]
    return np.ascontiguousarray(
        W.reshape(4, 128, 512).transpose(1, 0, 2).reshape(128, 2048)).astype(BF)


def _pack_e3(W):  # [512, 256] -> [128, 4*256]
    return np.ascontiguousarray(
        W.reshape(4, 128, 256).transpose(1, 0, 2).reshape(128, 1024)).astype(BF)


def _pack_hd(W):  # [256, 3] -> [128, 2*3]
    out = np.zeros((128, 6), np.float32)
    for kc in range(2):
        out[:, kc * 3:(kc + 1) * 3] = W[kc * 128:(kc + 1) * 128, :]
    return out.astype(BF)


def _pack_mot(params):
    """Block-diagonal motor weights: [128, 22*108].

    K-chunk (module k, kc) at cols (2*MIDX[k]+kc)*108; within the 108 columns,
    module k's mean cols land at ACT_OFF[k].. and std cols at 64+ACT_OFF[k]..
    (row-64 start keeps the std block partition-base 32-aligned on chip).
    """
    out = np.zeros((128, 22 * MOT_M), np.float32)
    for k in MODULES:
        W = np.asarray(params["motor"][k][0], np.float32) * MOTOR_SCALE
        a = ACTD[k]
        for kc in range(2):
            blk = W[kc * 128:(kc + 1) * 128, :]
            base = (2 * MIDX[k] + kc) * MOT_M
            out[:, base + ACT_OFF[k]: base + ACT_OFF[k] + a] = blk[:, :a]
            out[:, base + 64 + ACT_OFF[k]: base + 64 + ACT_OFF[k] + a] = blk[:, a:]
    return out.astype(BF)


_NC_CACHE = {}
LAST_RESULT = None


def kernel(obs, raw_action, params):
    global LAST_RESULT
    obs = {k: np.asarray(v, np.float32) for k, v in obs.items()}
    raw = {k: np.asarray(v, np.float32) for k, v in raw_action.items()}

    def P(x):
        return np.asarray(x, np.float32)

    zero_bias = all([
        all(not P(params["inp"][k][1]).any() for k in MODULES),
        all(not P(params["aff"][k][1]).any() for k in NONROOT),
        all(not P(params["eff"][k][1]).any() for k in NONROOT),
        all(not P(p[1]).any() for p in params["enc"]),
    ])

    if zero_bias not in _NC_CACHE:
        _NC_CACHE[zero_bias] = _build_nc(zero_bias)
    nc = _NC_CACHE[zero_bias]

    # ---- pack host arrays ------------------------------------------------
    obsT = np.zeros((PAD_OBS, B), BF)
    for k in MODULES:
        ci, slot = MOD_CHUNK[k]
        obsT[ci * 128 + slot: ci * 128 + slot + OBS[k], :] = obs[k].T.astype(BF)
    rawT = np.ascontiguousarray(
        np.concatenate([raw[k] for k in MODULES], axis=1).T)        # [44, B]

    w_inp = np.zeros((PAD_OBS, H), np.float32)
    for k in MODULES:
        ci, slot = MOD_CHUNK[k]
        w_inp[ci * 128 + slot: ci * 128 + slot + OBS[k], :] = P(params["inp"][k][0])
    w_inp = w_inp.astype(BF)
    msg_mats = [P(params["aff"][k][0]) for k in NONROOT] + \
               [P(params["eff"][k][0]) for k in NONROOT]
    w_msg = _pack_msg(msg_mats)
    w_mot = _pack_mot(params)
    w_e1_orig = P(params["enc"][0][0])                               # [270, 512]
    w_e1 = np.zeros((PAD_OBS, 512), np.float32)
    for k in MODULES:
        ci, slot = MOD_CHUNK[k]
        w_e1[ci * 128 + slot: ci * 128 + slot + OBS[k], :] = \
            w_e1_orig[OBS_OFF[k]:OBS_OFF[k] + OBS[k], :]
    w_e1 = w_e1.astype(BF)
    w_e2 = _pack_e2(P(params["enc"][1][0]))
    w_e3 = _pack_e3(P(params["enc"][2][0]))
    w_hd = _pack_hd(np.concatenate([P(params["heads"][r][0]) for r in RK], axis=1))

    b_mot = np.zeros((MOT_M, 1), np.float32)
    for k in MODULES:
        bv = P(params["motor"][k][1]) * MOTOR_SCALE
        a = ACTD[k]
        b_mot[ACT_OFF[k]:ACT_OFF[k] + a, 0] = bv[:a]
        b_mot[64 + ACT_OFF[k]:64 + ACT_OFF[k] + a, 0] = bv[a:]
    b_hd = np.stack([P(params["heads"][r][1]) for r in RK]).reshape(3, 1)

    shared = {
        "w_inp": w_inp, "w_msg": w_msg, "w_mot": w_mot, "w_e1": w_e1,
        "w_e2": w_e2, "w_e3": w_e3, "w_hd": w_hd, "b_mot": b_mot, "b_hd": b_hd,
    }
    if not zero_bias:
        b_inp = np.zeros((128, 22), np.float32)
        for k in MODULES:
            bv = P(params["inp"][k][1])
            for mc in range(2):
                b_inp[:, MIDX[k] * 2 + mc] = bv[mc * 128:(mc + 1) * 128]
        b_msg = np.zeros((128, 40), np.float32)
        for kind in ("aff", "eff"):
            for k in NONROOT:
                mi = MSG_IDX[(kind, k)]
                bv = P(params[kind][k][1])
                for mc in range(2):
                    b_msg[:, mi * 2 + mc] = bv[mc * 128:(mc + 1) * 128]
        b_e1 = P(params["enc"][0][1]).reshape(4, 128).T.copy()
        b_e2 = P(params["enc"][1][1]).reshape(4, 128).T.copy()
        b_e3 = P(params["enc"][2][1]).reshape(2, 128).T.copy()
        shared.update({"b_inp": b_inp, "b_msg": b_msg, "b_e1": b_e1,
                       "b_e2": b_e2, "b_e3": b_e3})

    in_maps = []
    for i in range(N_CORES):
        m = dict(shared)
        m["obsT"] = np.ascontiguousarray(obsT[:, i * BS:(i + 1) * BS])
        m["rawT"] = np.ascontiguousarray(rawT[:, i * BS:(i + 1) * BS])
        in_maps.append(m)

    trace = bool(int(os.environ.get("KERNEL_TRACE", "0")))
    res = run_bass_kernel_spmd(nc, in_maps, list(range(N_CORES)), trace=trace)
    LAST_RESULT = res

    out = np.empty((B, 49), np.float32)
    ent_sum = 0.0
    for i in range(N_CORES):
        r = res.results[i]
        sl = slice(i * BS, (i + 1) * BS)
        out[sl, 0:44] = r["out_act"].T
        out[sl, 44] = r["out_ll"][0]
        out[sl, 45:48] = r["out_vals"].T
        ent_sum += float(r["out_ent"].sum())
    ent = 22.0 * (1.0 + LOG2PI) + ent_sum / B
    out[:, 48] = -ENT_W * ent
    return out


# revision 25
# speedup vs baseline: 1.4949x; 1.2641x over previous
"""NerveNet MLP critic network — Trainium2 Bass kernel (8-core data parallel).

Layout strategy: everything runs feature-major (features on SBUF partitions,
batch on the free axis), so every GEMM is `out = W.T @ xT` with the weight as
the PE stationary operand and the transposed activations streaming.  Inputs
are transposed on the host; the output is transposed back on the host.

GEMM operands are bf16 (weight loads hide under FWL, matmuls issue
back-to-back at N cycles); all accumulation is fp32 in PSUM, and the entire
tanh-Normal log-likelihood / entropy path runs in fp32.  Swish eviction
PSUM->SBUF is fused into ScalarE activation instructions spanning 4 PSUM
banks.  The ln/exp distribution math is deferred to a single post-loop phase
so the ScalarE activation-table set only switches once.
"""

import os
import sys
import types
import math

sys.path.insert(0, "/opt/trn_rl_repo")

import numpy as np
import ml_dtypes

BF = ml_dtypes.bfloat16

# ---------------------------------------------------------------------------
# antenv.axon_hooks shim (enables NTFF profiling under axon in this image)
# ---------------------------------------------------------------------------
if "antenv.axon_hooks" not in sys.modules:
    try:
        import antenv  # noqa: F401

        _mod = types.ModuleType("antenv.axon_hooks")
        _hook_box = [None]
        _mod.set_axon_ntff_profile_hook = lambda h: _hook_box.__setitem__(0, h)
        _mod.get_axon_ntff_profile_hook = lambda: _hook_box[0]
        sys.modules["antenv.axon_hooks"] = _mod
        from trn_agent_boot.trn_boot import _ntff_profile_via_ctypes

        _mod.set_axon_ntff_profile_hook(
            _ntff_profile_via_ctypes("/opt/axon/libaxon_pjrt.so")
        )
    except Exception:
        pass

import concourse.bacc as bacc
import concourse.mybir as mybir
from concourse.tile import TileContext
from concourse.bass_utils import run_bass_kernel_spmd

F32 = mybir.dt.float32
BF16 = mybir.dt.bfloat16
AF = mybir.ActivationFunctionType
ALU = mybir.AluOpType

# ---------------------------------------------------------------------------
# Problem constants (hardcoded per the task contract)
# ---------------------------------------------------------------------------
MODULES = ["root", "torso", "head", "arm_L", "arm_R", "hand_L", "hand_R",
           "leg_L", "leg_R", "foot_L", "foot_R"]
OBS = {"root": 60, "torso": 30, "head": 20, "arm_L": 25, "arm_R": 25,
       "hand_L": 15, "hand_R": 15, "leg_L": 25, "leg_R": 25,
       "foot_L": 15, "foot_R": 15}
ACTD = {"root": 6, "torso": 3, "head": 3, "arm_L": 4, "arm_R": 4,
        "hand_L": 5, "hand_R": 5, "leg_L": 4, "leg_R": 4,
        "foot_L": 3, "foot_R": 3}
RK = ["tracking", "control", "alive"]
B = 32768
H = 256
MIN_STD = 0.1
ENT_W = 0.01
MOTOR_SCALE = 1.0
TOTAL_OBS = 270
TOTAL_ACT = 44

N_CORES = 8
BS = B // N_CORES          # 4096 rows per core
NT = 512                   # batch-tile (free dim per matmul)
NTILES = BS // NT          # 8

LOG2PI = float(np.log(2.0 * np.pi))

OBS_OFF = {}
_o = 0
for _k in MODULES:
    OBS_OFF[_k] = _o
    _o += OBS[_k]

ACT_OFF = {}
_o = 0
for _k in MODULES:
    ACT_OFF[_k] = _o
    _o += ACTD[_k]

# Obs rows are scattered into padded 128-row chunks; every module's rows
# start at a PE-quadrant-legal partition offset (0/32/64).  Padding rows are
# zero in both the obs and the critic L1 weight, so full-128 K-chunks stay
# exact for the critic while per-module slices drive the input layers.
CHUNKS = [(("root", 0), ("torso", 64)),
          (("head", 0), ("arm_L", 32), ("arm_R", 64)),
          (("hand_L", 0), ("hand_R", 32), ("leg_L", 64)),
          (("leg_R", 0), ("foot_L", 32), ("foot_R", 64))]
N_CHUNKS = len(CHUNKS)
PAD_OBS = N_CHUNKS * 128  # 512
MOD_CHUNK = {}
for _ci, _c in enumerate(CHUNKS):
    for _m, _slot in _c:
        assert _slot + OBS[_m] <= 128
        MOD_CHUNK[_m] = (_ci, _slot)

NONROOT = [m for m in MODULES if m != "root"]
MSG_IDX = {}
for _i, _m in enumerate(NONROOT):
    MSG_IDX[("aff", _m)] = _i
    MSG_IDX[("eff", _m)] = 10 + _i

MIDX = {m: i for i, m in enumerate(MODULES)}

MOT_M = 108  # motor psum rows: mean at 0:44, std_p at 64:108 (32-aligned)


def _build_nc(zero_bias: bool):
    nc = bacc.Bacc(None, target_bir_lowering=False)

    # ---- DRAM parameters -------------------------------------------------
    obsT = nc.declare_dram_parameter("obsT", [PAD_OBS, BS], BF16, isOutput=False)
    rawT = nc.declare_dram_parameter("rawT", [TOTAL_ACT, BS], F32, isOutput=False)
    w_inp = nc.declare_dram_parameter("w_inp", [PAD_OBS, H], BF16, isOutput=False)
    w_msg = nc.declare_dram_parameter("w_msg", [128, 20 * 512], BF16, isOutput=False)
    w_mot = nc.declare_dram_parameter("w_mot", [128, 22 * MOT_M], BF16, isOutput=False)
    w_e1 = nc.declare_dram_parameter("w_e1", [PAD_OBS, 512], BF16, isOutput=False)
    w_e2 = nc.declare_dram_parameter("w_e2", [128, 2048], BF16, isOutput=False)
    w_e3 = nc.declare_dram_parameter("w_e3", [128, 1024], BF16, isOutput=False)
    w_hd = nc.declare_dram_parameter("w_hd", [128, 6], BF16, isOutput=False)
    b_mot = nc.declare_dram_parameter("b_mot", [MOT_M, 1], F32, isOutput=False)
    b_hd = nc.declare_dram_parameter("b_hd", [3, 1], F32, isOutput=False)
    if not zero_bias:
        b_inp = nc.declare_dram_parameter("b_inp", [128, 22], F32, isOutput=False)
        b_msg = nc.declare_dram_parameter("b_msg", [128, 40], F32, isOutput=False)
        b_e1 = nc.declare_dram_parameter("b_e1", [128, 4], F32, isOutput=False)
        b_e2 = nc.declare_dram_parameter("b_e2", [128, 4], F32, isOutput=False)
        b_e3 = nc.declare_dram_parameter("b_e3", [128, 2], F32, isOutput=False)

    out_act = nc.declare_dram_parameter("out_act", [TOTAL_ACT, BS], F32, isOutput=True)
    out_ll = nc.declare_dram_parameter("out_ll", [1, BS], F32, isOutput=True)
    out_vals = nc.declare_dram_parameter("out_vals", [3, BS], F32, isOutput=True)
    out_ent = nc.declare_dram_parameter("out_ent", [TOTAL_ACT, 2], F32, isOutput=True)

    with TileContext(nc) as tc:
        with (
            tc.tile_pool(name="spool", bufs=1) as spool,
            tc.tile_pool(name="ppool", bufs=2, space="PSUM") as ppool,
        ):
            # ---- batch-lifetime staging tiles (all fp32) ----------------
            mstd_sb = spool.tile([MOT_M, BS], F32, tag="mstd")
            # values rows 0:3, loglik row 32
            vl_sb = spool.tile([33, BS], F32, tag="vl")
            act_sb = spool.tile([TOTAL_ACT, BS], F32, tag="act")   # tanh(raw)
            d2_sb = spool.tile([TOTAL_ACT, BS], F32, tag="d2")     # (raw-mean)^2
            u_sb = spool.tile([TOTAL_ACT, BS], F32, tag="u")       # tanh^2
            entp_sb = spool.tile([TOTAL_ACT, 2], F32, tag="entp")
            ones_sb = spool.tile([TOTAL_ACT, 1], F32, tag="ones")
            nc.vector.memset(ones_sb[:], 1.0)
            c_minstd = spool.tile([TOTAL_ACT, 1], F32, tag="cmin")
            nc.vector.memset(c_minstd[:], MIN_STD)
            c_eps = spool.tile([TOTAL_ACT, 1], F32, tag="ceps")
            nc.vector.memset(c_eps[:], 1.0 + 1e-6)

            def mm(out, lhsT, rhs, start, stop):
                nc.tensor.matmul(out, lhsT, rhs, start=start, stop=stop)

            # ================= phase A: network + motor ==================
            with (
                tc.tile_pool(name="wpool", bufs=1) as wpool,
                tc.tile_pool(name="obspool", bufs=2) as obspool,
                tc.tile_pool(name="rawpool", bufs=2) as rawpool,
                tc.tile_pool(name="xpool", bufs=2) as xpool,
                tc.tile_pool(name="tpool", bufs=2) as tpool,
                tc.tile_pool(name="rpool", bufs=1) as rpool,
                tc.tile_pool(name="hpool", bufs=1) as hpool,
            ):
                # ---- persistent weight tiles ----------------------------
                w_inp_sb = []
                w_e1_sb = []
                for ci in range(N_CHUNKS):
                    ti = wpool.tile([128, H], BF16, tag=f"winp{ci}")
                    nc.sync.dma_start(out=ti[:], in_=w_inp[ci * 128:(ci + 1) * 128, :])
                    w_inp_sb.append(ti)
                for ci in range(N_CHUNKS):
                    w_e1_sb.append(wpool.tile([128, 512], BF16, tag=f"we1{ci}",
                                              name=f"we1_{ci}"))
                w_msg_sb = wpool.tile([128, 20 * 512], BF16, tag="wmsg")
                w_mot_sb = wpool.tile([128, 22 * MOT_M], BF16, tag="wmot")
                w_e2_sb = wpool.tile([128, 2048], BF16, tag="we2")
                w_e3_sb = wpool.tile([128, 1024], BF16, tag="we3")
                w_hd_sb = wpool.tile([128, 6], BF16, tag="whd")
                b_mot_sb = wpool.tile([MOT_M, 1], F32, tag="bmot")
                b_hd_sb = wpool.tile([3, 1], F32, tag="bhd")
                if not zero_bias:
                    b_inp_sb = wpool.tile([128, 22], F32, tag="binp")
                    b_msg_sb = wpool.tile([128, 40], F32, tag="bmsg")
                    b_e1_sb = wpool.tile([128, 4], F32, tag="be1")
                    b_e2_sb = wpool.tile([128, 4], F32, tag="be2")
                    b_e3_sb = wpool.tile([128, 2], F32, tag="be3")

                def load_big_weights():
                    # issued after tile 0's obs DMAs so the first input tiles
                    # land immediately; these stream in behind them.
                    if not zero_bias:
                        nc.sync.dma_start(out=b_inp_sb[:], in_=b_inp[:])
                        nc.sync.dma_start(out=b_msg_sb[:], in_=b_msg[:])
                        nc.sync.dma_start(out=b_e1_sb[:], in_=b_e1[:])
                        nc.sync.dma_start(out=b_e2_sb[:], in_=b_e2[:])
                        nc.sync.dma_start(out=b_e3_sb[:], in_=b_e3[:])
                    for q in range(8):
                        nc.sync.dma_start(out=w_msg_sb[:, q * 1280:(q + 1) * 1280],
                                          in_=w_msg[:, q * 1280:(q + 1) * 1280])
                    for ci in range(N_CHUNKS):
                        te = w_e1_sb[ci]
                        nc.sync.dma_start(out=te[:],
                                          in_=w_e1[ci * 128:(ci + 1) * 128, :])
                    nc.sync.dma_start(out=w_mot_sb[:], in_=w_mot[:])
                    nc.sync.dma_start(out=w_e2_sb[:], in_=w_e2[:])
                    nc.sync.dma_start(out=w_e3_sb[:], in_=w_e3[:])
                    nc.sync.dma_start(out=w_hd_sb[:], in_=w_hd[:])
                    nc.sync.dma_start(out=b_mot_sb[:], in_=b_mot[:])
                    nc.sync.dma_start(out=b_hd_sb[:], in_=b_hd[:])

                for t in range(NTILES):
                    ct = t * NT

                    # xall: module k at cols [k*2*NT, (k+1)*2*NT); within a
                    # module: cols 0:NT = features 0:128, NT:2NT = 128:256
                    xall = xpool.tile([128, 11 * 2 * NT], BF16, tag="xall")

                    def xsl(k, kc=None):
                        base = MIDX[k] * 2 * NT
                        if kc is None:
                            return xall[:, base:base + 2 * NT]
                        return xall[:, base + kc * NT: base + (kc + 1) * NT]

                    def evict_swish(ps_ap, dst_ap, nblk, bias_cols):
                        if zero_bias:
                            nc.scalar.activation(dst_ap, ps_ap, AF.Silu)
                        else:
                            for i in range(nblk):
                                bt, bc = bias_cols[i]
                                nc.scalar.activation(
                                    dst_ap[:, i * NT:(i + 1) * NT],
                                    ps_ap[:, i * NT:(i + 1) * NT],
                                    AF.Silu, bias=bt[:, bc:bc + 1])

                    # -- load obs chunks + raw slice ----------------------
                    obs_t = []
                    for ci in range(N_CHUNKS):
                        ob = obspool.tile([128, NT], BF16, tag=f"obs{ci}")
                        nc.sync.dma_start(
                            out=ob[:],
                            in_=obsT[ci * 128:(ci + 1) * 128, ct:ct + NT])
                        obs_t.append(ob)
                    raw_t = rawpool.tile([TOTAL_ACT, NT], F32, tag="raw")
                    nc.sync.dma_start(out=raw_t[:], in_=rawT[:, ct:ct + NT])
                    if t == 0:
                        load_big_weights()
                    # tanh(raw) lives in the silu table set -> in-loop
                    nc.scalar.activation(act_sb[:, ct:ct + NT], raw_t[:], AF.Tanh)
                    nc.vector.tensor_mul(u_sb[:, ct:ct + NT],
                                         act_sb[:, ct:ct + NT],
                                         act_sb[:, ct:ct + NT])

                    # -- input layers: pairs of adjacent modules ----------
                    pairs = [("root", "torso"), ("head", "arm_L"),
                             ("arm_R", "hand_L"), ("hand_R", "leg_L"),
                             ("leg_R", "foot_L"), ("foot_R",)]
                    for pr in pairs:
                        pw = len(pr) * 2 * NT
                        ps = ppool.tile([128, 2048], F32, tag="ps")
                        for j, k in enumerate(pr):
                            ci, ro = MOD_CHUNK[k]
                            d = OBS[k]
                            for mc in range(2):
                                mm(ps[:, j * 2 * NT + mc * NT:
                                      j * 2 * NT + (mc + 1) * NT],
                                   w_inp_sb[ci][ro:ro + d, mc * 128:(mc + 1) * 128],
                                   obs_t[ci][ro:ro + d, :],
                                   start=True, stop=True)
                        base = MIDX[pr[0]] * 2 * NT
                        bias_cols = []
                        if not zero_bias:
                            for k in pr:
                                for mc in range(2):
                                    bias_cols.append((b_inp_sb, MIDX[k] * 2 + mc))
                        evict_swish(ps[:, 0:pw], xall[:, base:base + pw],
                                    len(pr) * 2, bias_cols)

                    # -- message-passing pair: swish(W.T @ src) -> tmp ----
                    def msg_pair(kind, mods, srcs):
                        ps = ppool.tile([128, 2048], F32, tag="ps")
                        for j, (k, src) in enumerate(zip(mods, srcs)):
                            mi = MSG_IDX[(kind, k)]
                            wb = mi * 512
                            for mc in range(2):
                                for kc in range(2):
                                    mm(ps[:, j * 2 * NT + mc * NT:
                                          j * 2 * NT + (mc + 1) * NT],
                                       w_msg_sb[:, wb + kc * 256 + mc * 128:
                                                wb + kc * 256 + (mc + 1) * 128],
                                       src[:, kc * NT:(kc + 1) * NT],
                                       start=(kc == 0), stop=(kc == 1))
                        tmp = tpool.tile([128, 2048], BF16, tag="msgtmp")
                        w = len(mods) * 2 * NT
                        bias_cols = []
                        if not zero_bias:
                            for k in mods:
                                mi = MSG_IDX[(kind, k)]
                                for mc in range(2):
                                    bias_cols.append((b_msg_sb, mi * 2 + mc))
                        evict_swish(ps[:, 0:w], tmp[:, 0:w], len(mods) * 2, bias_cols)
                        return tmp

                    # -- critic stages as closures, interleaved below -----
                    crit = {}

                    def enc1():
                        ps1 = ppool.tile([128, 2048], F32, tag="ps")
                        for mc in range(4):
                            for ci in range(N_CHUNKS):
                                mm(ps1[:, mc * NT:(mc + 1) * NT],
                                   w_e1_sb[ci][:, mc * 128:(mc + 1) * 128],
                                   obs_t[ci][:],
                                   start=(ci == 0), stop=(ci == N_CHUNKS - 1))
                        h1 = hpool.tile([128, 2048], BF16, tag="h1")
                        if zero_bias:
                            nc.scalar.activation(h1[:], ps1[:], AF.Silu)
                        else:
                            for mc in range(4):
                                nc.scalar.activation(h1[:, mc * NT:(mc + 1) * NT],
                                                     ps1[:, mc * NT:(mc + 1) * NT],
                                                     AF.Silu,
                                                     bias=b_e1_sb[:, mc:mc + 1])
                        crit["h1"] = h1

                    def enc2():
                        h1 = crit["h1"]
                        ps2 = ppool.tile([128, 2048], F32, tag="ps")
                        for mc in range(4):
                            for kc in range(4):
                                mm(ps2[:, mc * NT:(mc + 1) * NT],
                                   w_e2_sb[:, kc * 512 + mc * 128:
                                           kc * 512 + (mc + 1) * 128],
                                   h1[:, kc * NT:(kc + 1) * NT],
                                   start=(kc == 0), stop=(kc == 3))
                        h2 = hpool.tile([128, 2048], BF16, tag="h2")
                        if zero_bias:
                            nc.scalar.activation(h2[:], ps2[:], AF.Silu)
                        else:
                            for mc in range(4):
                                nc.scalar.activation(h2[:, mc * NT:(mc + 1) * NT],
                                                     ps2[:, mc * NT:(mc + 1) * NT],
                                                     AF.Silu,
                                                     bias=b_e2_sb[:, mc:mc + 1])
                        crit["h2"] = h2

                    def enc3():
                        h2 = crit["h2"]
                        ps3 = ppool.tile([128, 2048], F32, tag="ps")
                        for mc in range(2):
                            for kc in range(4):
                                mm(ps3[:, mc * NT:(mc + 1) * NT],
                                   w_e3_sb[:, kc * 256 + mc * 128:
                                           kc * 256 + (mc + 1) * 128],
                                   h2[:, kc * NT:(kc + 1) * NT],
                                   start=(kc == 0), stop=(kc == 3))
                        h3 = hpool.tile([128, 1024], BF16, tag="h3")
                        if zero_bias:
                            nc.scalar.activation(h3[:], ps3[:, 0:1024], AF.Silu)
                        else:
                            for mc in range(2):
                                nc.scalar.activation(h3[:, mc * NT:(mc + 1) * NT],
                                                     ps3[:, mc * NT:(mc + 1) * NT],
                                                     AF.Silu,
                                                     bias=b_e3_sb[:, mc:mc + 1])
                        crit["h3"] = h3

                    def heads():
                        h3 = crit["h3"]
                        psv = crit["psm"]  # share the motor psum tile, bank 1
                        for kc in range(2):
                            mm(psv[0:3, NT:2 * NT], w_hd_sb[:, kc * 3:(kc + 1) * 3],
                               h3[:, kc * NT:(kc + 1) * NT],
                               start=(kc == 0), stop=(kc == 1))
                        nc.vector.tensor_scalar_add(vl_sb[0:3, ct:ct + NT],
                                                    psv[0:3, NT:2 * NT],
                                                    b_hd_sb[0:3, 0:1])

                    enc1()

                    # afferent leaf -> mid (merged adds over adjacent dst)
                    tmp = msg_pair("aff", ["hand_L", "hand_R"],
                                   [xsl("hand_L"), xsl("hand_R")])
                    d0 = MIDX["arm_L"] * 2 * NT
                    nc.vector.tensor_add(xall[:, d0:d0 + 4 * NT],
                                         xall[:, d0:d0 + 4 * NT], tmp[:, 0:4 * NT])
                    tmp = msg_pair("aff", ["foot_L", "foot_R"],
                                   [xsl("foot_L"), xsl("foot_R")])
                    d0 = MIDX["leg_L"] * 2 * NT
                    nc.vector.tensor_add(xall[:, d0:d0 + 4 * NT],
                                         xall[:, d0:d0 + 4 * NT], tmp[:, 0:4 * NT])
                    # afferent mid -> root: 3 pairs, tree-reduced
                    rts = []
                    for pi, mods in enumerate((["arm_L", "arm_R"],
                                               ["leg_L", "leg_R"],
                                               ["torso", "head"])):
                        tmp = msg_pair("aff", mods, [xsl(mods[0]), xsl(mods[1])])
                        rt = rpool.tile([128, 2 * NT], BF16, tag=f"rt{pi}")
                        nc.vector.tensor_add(rt[:], tmp[:, 0:2 * NT],
                                             tmp[:, 2 * NT:4 * NT])
                        rts.append(rt)
                    nc.vector.tensor_add(rts[0][:], rts[0][:], rts[1][:])
                    nc.vector.tensor_add(rts[0][:], rts[0][:], rts[2][:])
                    nc.vector.tensor_add(xsl("root"), xsl("root"), rts[0][:])

                    enc2()

                    # efferent root wave
                    for mods in (["torso", "head"], ["arm_L", "arm_R"],
                                 ["leg_L", "leg_R"]):
                        tmp = msg_pair("eff", mods, [xsl("root"), xsl("root")])
                        d0 = MIDX[mods[0]] * 2 * NT
                        nc.vector.tensor_add(xall[:, d0:d0 + 4 * NT],
                                             xall[:, d0:d0 + 4 * NT],
                                             tmp[:, 0:4 * NT])
                    enc3()

                    # efferent leaf wave
                    tmp = msg_pair("eff", ["hand_L", "hand_R"],
                                   [xsl("arm_L"), xsl("arm_R")])
                    d0 = MIDX["hand_L"] * 2 * NT
                    nc.vector.tensor_add(xall[:, d0:d0 + 4 * NT],
                                         xall[:, d0:d0 + 4 * NT], tmp[:, 0:4 * NT])
                    tmp = msg_pair("eff", ["foot_L", "foot_R"],
                                   [xsl("leg_L"), xsl("leg_R")])
                    d0 = MIDX["foot_L"] * 2 * NT
                    nc.vector.tensor_add(xall[:, d0:d0 + 4 * NT],
                                         xall[:, d0:d0 + 4 * NT], tmp[:, 0:4 * NT])

                    # -- motor heads: 22 accumulating matmuls -> [108, NT] -
                    psm = ppool.tile([128, 2048], F32, tag="ps")
                    crit["psm"] = psm
                    nmm = 0
                    for k in MODULES:
                        for kc in range(2):
                            blk = (2 * MIDX[k] + kc) * MOT_M
                            mm(psm[0:MOT_M, 0:NT], w_mot_sb[:, blk:blk + MOT_M],
                               xsl(k, kc), start=(nmm == 0), stop=(nmm == 21))
                            nmm += 1
                    nc.vector.tensor_scalar_add(mstd_sb[:, ct:ct + NT],
                                                psm[0:MOT_M, 0:NT],
                                                b_mot_sb[0:MOT_M, 0:1])
                    # d2 = (raw - mean)^2, computed in-loop (fp32)
                    nc.vector.tensor_sub(d2_sb[:, ct:ct + NT], raw_t[:],
                                         mstd_sb[0:44, ct:ct + NT])
                    nc.vector.tensor_mul(d2_sb[:, ct:ct + NT],
                                         d2_sb[:, ct:ct + NT],
                                         d2_sb[:, ct:ct + NT])

                    heads()

            # ================= phase B: distribution math ================
            # Two column halves, software-pipelined so the serial exp/ln ACT
            # chain of one half overlaps the DVE work of the other.
            with tc.tile_pool(name="dpool", bufs=1) as dpool:
                HB = BS // 2
                bufA = [dpool.tile([TOTAL_ACT, HB], F32, tag=f"bufA{h}",
                                   name=f"bufA{h}") for h in range(2)]
                bufB = [dpool.tile([TOTAL_ACT, HB], F32, tag=f"bufB{h}",
                                   name=f"bufB{h}") for h in range(2)]
                bufC = [dpool.tile([TOTAL_ACT, HB], F32, tag=f"bufC{h}",
                                   name=f"bufC{h}") for h in range(2)]
                for h in range(2):
                    c0 = h * HB
                    mean = mstd_sb[0:44, c0:c0 + HB]
                    stdp = mstd_sb[64:64 + 44, c0:c0 + HB]
                    A, Bb, C = bufA[h], bufB[h], bufC[h]
                    nc.scalar.activation(A[:], stdp, AF.Exp)
                    nc.scalar.activation(Bb[:], A[:], AF.Ln, bias=1.0)
                    nc.scalar.activation(A[:], Bb[:], AF.Ln, bias=c_minstd[:],
                                         accum_out=entp_sb[:, h:h + 1])
                    nc.scalar.activation(Bb[:], A[:], AF.Exp, scale=-2.0)
                    nc.vector.tensor_mul(C[:], d2_sb[:, c0:c0 + HB], Bb[:])
                    nc.vector.scalar_tensor_tensor(Bb[:], C[:], -0.5, A[:],
                                                   op0=ALU.mult,
                                                   op1=ALU.subtract)
                    nc.scalar.activation(A[:], u_sb[:, c0:c0 + HB], AF.Ln,
                                         scale=-1.0, bias=c_eps[:])
                    nc.vector.tensor_sub(C[:], Bb[:], A[:])
                    for tt in range(NTILES // 2):
                        ct = tt * NT
                        pl = ppool.tile([128, 2048], F32, tag="ps")
                        nc.tensor.matmul(pl[0:1, 0:NT], ones_sb[:],
                                         C[:, ct:ct + NT], start=True, stop=True)
                        nc.vector.tensor_scalar_add(
                            vl_sb[32:33, c0 + ct:c0 + ct + NT], pl[0:1, 0:NT],
                            float(-22.0 * LOG2PI))

                # ---- output DMAs ----------------------------------------
                nc.sync.dma_start(out=out_act[:], in_=act_sb[:])
                nc.sync.dma_start(out=out_ll[:], in_=vl_sb[32:33, :])
                nc.sync.dma_start(out=out_vals[:], in_=vl_sb[0:3, :])
                nc.sync.dma_start(out=out_ent[:], in_=entp_sb[:])

    nc.finalize()
    return nc


# ---------------------------------------------------------------------------
# Host-side weight packing
# ---------------------------------------------------------------------------

def _pack_msg(Ws):
    """20 [256,256] matrices -> [128, 20*512]; (kc,mc) block at j*512+kc*256+mc*128."""
    out = np.zeros((128, len(Ws) * 512), np.float32)
    for j, W in enumerate(Ws):
        blk = W.reshape(2, 128, 2, 128).transpose(1, 0, 2, 3).reshape(128, 512)
        out[:, j * 512:(j + 1) * 512] = blk
    return out.astype(BF)


def _pack_e2(W):  # [512, 512] -> [128, 4*512]
    return np.ascontiguousarray(
        W.reshape(4, 128, 512).transpose(1, 0, 2).reshape(128, 2048)).astype(BF)


def _pack_e3(W):  # [512, 256] -> [128, 4*256]
    return np.ascontiguousarray(
        W.reshape(4, 128, 256).transpose(1, 0, 2).reshape(128, 1024)).astype(BF)


def _pack_hd(W):  # [256, 3] -> [128, 2*3]
    out = np.zeros((128, 6), np.float32)
    for kc in range(2):
        out[:, kc * 3:(kc + 1) * 3] = W[kc * 128:(kc + 1) * 128, :]
    return out.astype(BF)


def _pack_mot(params):
    """Block-diagonal motor weights: [128, 22*108].

    K-chunk (module k, kc) at cols (2*MIDX[k]+kc)*108; within the 108 columns,
    module k's mean cols land at ACT_OFF[k].. and std cols at 64+ACT_OFF[k]..
    (row-64 start keeps the std block partition-base 32-aligned on chip).
    """
    out = np.zeros((128, 22 * MOT_M), np.float32)
    for k in MODULES:
        W = np.asarray(params["motor"][k][0], np.float32) * MOTOR_SCALE
        a = ACTD[k]
        for kc in range(2):
            blk = W[kc * 128:(kc + 1) * 128, :]
            base = (2 * MIDX[k] + kc) * MOT_M
            out[:, base + ACT_OFF[k]: base + ACT_OFF[k] + a] = blk[:, :a]
            out[:, base + 64 + ACT_OFF[k]: base + 64 + ACT_OFF[k] + a] = blk[:, a:]
    return out.astype(BF)


_NC_CACHE = {}
LAST_RESULT = None


def kernel(obs, raw_action, params):
    global LAST_RESULT
    obs = {k: np.asarray(v, np.float32) for k, v in obs.items()}
    raw = {k: np.asarray(v, np.float32) for k, v in raw_action.items()}

    def P(x):
        return np.asarray(x, np.float32)

    zero_bias = all([
        all(not P(params["inp"][k][1]).any() for k in MODULES),
        all(not P(params["aff"][k][1]).any() for k in NONROOT),
        all(not P(params["eff"][k][1]).any() for k in NONROOT),
        all(not P(p[1]).any() for p in params["enc"]),
    ])

    if zero_bias not in _NC_CACHE:
        _NC_CACHE[zero_bias] = _build_nc(zero_bias)
    nc = _NC_CACHE[zero_bias]

    # ---- pack host arrays ------------------------------------------------
    obsT = np.zeros((PAD_OBS, B), BF)
    for k in MODULES:
        ci, slot = MOD_CHUNK[k]
        obsT[ci * 128 + slot: ci * 128 + slot + OBS[k], :] = obs[k].T.astype(BF)
    rawT = np.ascontiguousarray(
        np.concatenate([raw[k] for k in MODULES], axis=1).T)        # [44, B]

    w_inp = np.zeros((PAD_OBS, H), np.float32)
    for k in MODULES:
        ci, slot = MOD_CHUNK[k]
        w_inp[ci * 128 + slot: ci * 128 + slot + OBS[k], :] = P(params["inp"][k][0])
    w_inp = w_inp.astype(BF)
    msg_mats = [P(params["aff"][k][0]) for k in NONROOT] + \
               [P(params["eff"][k][0]) for k in NONROOT]
    w_msg = _pack_msg(msg_mats)
    w_mot = _pack_mot(params)
    w_e1_orig = P(params["enc"][0][0])                               # [270, 512]
    w_e1 = np.zeros((PAD_OBS, 512), np.float32)
    for k in MODULES:
        ci, slot = MOD_CHUNK[k]
        w_e1[ci * 128 + slot: ci * 128 + slot + OBS[k], :] = \
            w_e1_orig[OBS_OFF[k]:OBS_OFF[k] + OBS[k], :]
    w_e1 = w_e1.astype(BF)
    w_e2 = _pack_e2(P(params["enc"][1][0]))
    w_e3 = _pack_e3(P(params["enc"][2][0]))
    w_hd = _pack_hd(np.concatenate([P(params["heads"][r][0]) for r in RK], axis=1))

    b_mot = np.zeros((MOT_M, 1), np.float32)
    for k in MODULES:
        bv = P(params["motor"][k][1]) * MOTOR_SCALE
        a = ACTD[k]
        b_mot[ACT_OFF[k]:ACT_OFF[k] + a, 0] = bv[:a]
        b_mot[64 + ACT_OFF[k]:64 + ACT_OFF[k] + a, 0] = bv[a:]
    b_hd = np.stack([P(params["heads"][r][1]) for r in RK]).reshape(3, 1)

    shared = {
        "w_inp": w_inp, "w_msg": w_msg, "w_mot": w_mot, "w_e1": w_e1,
        "w_e2": w_e2, "w_e3": w_e3, "w_hd": w_hd, "b_mot": b_mot, "b_hd": b_hd,
    }
    if not zero_bias:
        b_inp = np.zeros((128, 22), np.float32)
        for k in MODULES:
            bv = P(params["inp"][k][1])
            for mc in range(2):
                b_inp[:, MIDX[k] * 2 + mc] = bv[mc * 128:(mc + 1) * 128]
        b_msg = np.zeros((128, 40), np.float32)
        for kind in ("aff", "eff"):
            for k in NONROOT:
                mi = MSG_IDX[(kind, k)]
                bv = P(params[kind][k][1])
                for mc in range(2):
                    b_msg[:, mi * 2 + mc] = bv[mc * 128:(mc + 1) * 128]
        b_e1 = P(params["enc"][0][1]).reshape(4, 128).T.copy()
        b_e2 = P(params["enc"][1][1]).reshape(4, 128).T.copy()
        b_e3 = P(params["enc"][2][1]).reshape(2, 128).T.copy()
        shared.update({"b_inp": b_inp, "b_msg": b_msg, "b_e1": b_e1,
                       "b_e2": b_e2, "b_e3": b_e3})

    in_maps = []
    for i in range(N_CORES):
        m = dict(shared)
        m["obsT"] = np.ascontiguousarray(obsT[:, i * BS:(i + 1) * BS])
        m["rawT"] = np.ascontiguousarray(rawT[:, i * BS:(i + 1) * BS])
        in_maps.append(m)

    trace = bool(int(os.environ.get("KERNEL_TRACE", "0")))
    res = run_bass_kernel_spmd(nc, in_maps, list(range(N_CORES)), trace=trace)
    LAST_RESULT = res

    out = np.empty((B, 49), np.float32)
    ent_sum = 0.0
    for i in range(N_CORES):
        r = res.results[i]
        sl = slice(i * BS, (i + 1) * BS)
        out[sl, 0:44] = r["out_act"].T
        out[sl, 44] = r["out_ll"][0]
        out[sl, 45:48] = r["out_vals"].T
        ent_sum += float(r["out_ent"].sum())
    ent = 22.0 * (1.0 + LOG2PI) + ent_sum / B
    out[:, 48] = -ENT_W * ent
    return out


# revision 28
# speedup vs baseline: 1.5206x; 1.0172x over previous
"""NerveNet MLP critic network — Trainium2 Bass kernel (8-core data parallel).

Layout strategy: everything runs feature-major (features on SBUF partitions,
batch on the free axis), so every GEMM is `out = W.T @ xT` with the weight as
the PE stationary operand and the transposed activations streaming.  Inputs
are transposed on the host; the output is transposed back on the host.

GEMM operands are bf16 (weight loads hide under FWL, matmuls issue
back-to-back at N cycles); all accumulation is fp32 in PSUM, and the entire
tanh-Normal log-likelihood / entropy path runs in fp32.  Swish eviction
PSUM->SBUF is fused into ScalarE activation instructions spanning 4 PSUM
banks.  The ln/exp distribution math is deferred to a single post-loop phase
so the ScalarE activation-table set only switches once.
"""

import os
import sys
import types
import math

sys.path.insert(0, "/opt/trn_rl_repo")

import numpy as np
import ml_dtypes

BF = ml_dtypes.bfloat16

# ---------------------------------------------------------------------------
# antenv.axon_hooks shim (enables NTFF profiling under axon in this image)
# ---------------------------------------------------------------------------
if "antenv.axon_hooks" not in sys.modules:
    try:
        import antenv  # noqa: F401

        _mod = types.ModuleType("antenv.axon_hooks")
        _hook_box = [None]
        _mod.set_axon_ntff_profile_hook = lambda h: _hook_box.__setitem__(0, h)
        _mod.get_axon_ntff_profile_hook = lambda: _hook_box[0]
        sys.modules["antenv.axon_hooks"] = _mod
        from trn_agent_boot.trn_boot import _ntff_profile_via_ctypes

        _mod.set_axon_ntff_profile_hook(
            _ntff_profile_via_ctypes("/opt/axon/libaxon_pjrt.so")
        )
    except Exception:
        pass

import concourse.bacc as bacc
import concourse.mybir as mybir
from concourse.tile import TileContext
from concourse.bass_utils import run_bass_kernel_spmd

F32 = mybir.dt.float32
BF16 = mybir.dt.bfloat16
AF = mybir.ActivationFunctionType
ALU = mybir.AluOpType

# ---------------------------------------------------------------------------
# Problem constants (hardcoded per the task contract)
# ---------------------------------------------------------------------------
MODULES = ["root", "torso", "head", "arm_L", "arm_R", "hand_L", "hand_R",
           "leg_L", "leg_R", "foot_L", "foot_R"]
OBS = {"root": 60, "torso": 30, "head": 20, "arm_L": 25, "arm_R": 25,
       "hand_L": 15, "hand_R": 15, "leg_L": 25, "leg_R": 25,
       "foot_L": 15, "foot_R": 15}
ACTD = {"root": 6, "torso": 3, "head": 3, "arm_L": 4, "arm_R": 4,
        "hand_L": 5, "hand_R": 5, "leg_L": 4, "leg_R": 4,
        "foot_L": 3, "foot_R": 3}
RK = ["tracking", "control", "alive"]
B = 32768
H = 256
MIN_STD = 0.1
ENT_W = 0.01
MOTOR_SCALE = 1.0
TOTAL_OBS = 270
TOTAL_ACT = 44

N_CORES = 8
BS = B // N_CORES          # 4096 rows per core
NT = 512                   # batch-tile (free dim per matmul)
NTILES = BS // NT          # 8

LOG2PI = float(np.log(2.0 * np.pi))

OBS_OFF = {}
_o = 0
for _k in MODULES:
    OBS_OFF[_k] = _o
    _o += OBS[_k]

ACT_OFF = {}
_o = 0
for _k in MODULES:
    ACT_OFF[_k] = _o
    _o += ACTD[_k]

# Obs rows are scattered into padded 128-row chunks; every module's rows
# start at a PE-quadrant-legal partition offset (0/32/64).  Padding rows are
# zero in both the obs and the critic L1 weight, so full-128 K-chunks stay
# exact for the critic while per-module slices drive the input layers.
CHUNKS = [(("root", 0), ("torso", 64)),
          (("head", 0), ("arm_L", 32), ("arm_R", 64)),
          (("hand_L", 0), ("hand_R", 32), ("leg_L", 64)),
          (("leg_R", 0), ("foot_L", 32), ("foot_R", 64))]
N_CHUNKS = len(CHUNKS)
PAD_OBS = N_CHUNKS * 128  # 512
MOD_CHUNK = {}
for _ci, _c in enumerate(CHUNKS):
    for _m, _slot in _c:
        assert _slot + OBS[_m] <= 128
        MOD_CHUNK[_m] = (_ci, _slot)

NONROOT = [m for m in MODULES if m != "root"]
MSG_IDX = {}
for _i, _m in enumerate(NONROOT):
    MSG_IDX[("aff", _m)] = _i
    MSG_IDX[("eff", _m)] = 10 + _i

MIDX = {m: i for i, m in enumerate(MODULES)}

MOT_M = 108  # motor psum rows: mean at 0:44, std_p at 64:108 (32-aligned)


def _build_nc(zero_bias: bool):
    nc = bacc.Bacc(None, target_bir_lowering=False)

    # ---- DRAM parameters -------------------------------------------------
    obsT = nc.declare_dram_parameter("obsT", [PAD_OBS, BS], BF16, isOutput=False)
    rawT = nc.declare_dram_parameter("rawT", [TOTAL_ACT, BS], F32, isOutput=False)
    w_inp = nc.declare_dram_parameter("w_inp", [PAD_OBS, H], BF16, isOutput=False)
    w_msg = nc.declare_dram_parameter("w_msg", [128, 20 * 512], BF16, isOutput=False)
    w_mot = nc.declare_dram_parameter("w_mot", [128, 22 * MOT_M], BF16, isOutput=False)
    w_e1 = nc.declare_dram_parameter("w_e1", [PAD_OBS, 512], BF16, isOutput=False)
    w_e2 = nc.declare_dram_parameter("w_e2", [128, 2048], BF16, isOutput=False)
    w_e3 = nc.declare_dram_parameter("w_e3", [128, 1024], BF16, isOutput=False)
    w_hd = nc.declare_dram_parameter("w_hd", [128, 6], BF16, isOutput=False)
    b_mot = nc.declare_dram_parameter("b_mot", [MOT_M, 1], F32, isOutput=False)
    b_hd = nc.declare_dram_parameter("b_hd", [3, 1], F32, isOutput=False)
    if not zero_bias:
        b_inp = nc.declare_dram_parameter("b_inp", [128, 22], F32, isOutput=False)
        b_msg = nc.declare_dram_parameter("b_msg", [128, 40], F32, isOutput=False)
        b_e1 = nc.declare_dram_parameter("b_e1", [128, 4], F32, isOutput=False)
        b_e2 = nc.declare_dram_parameter("b_e2", [128, 4], F32, isOutput=False)
        b_e3 = nc.declare_dram_parameter("b_e3", [128, 2], F32, isOutput=False)

    out_act = nc.declare_dram_parameter("out_act", [TOTAL_ACT, BS], F32, isOutput=True)
    out_ll = nc.declare_dram_parameter("out_ll", [1, BS], F32, isOutput=True)
    out_vals = nc.declare_dram_parameter("out_vals", [3, BS], F32, isOutput=True)
    out_ent = nc.declare_dram_parameter("out_ent", [TOTAL_ACT, 2], F32, isOutput=True)

    with TileContext(nc) as tc:
        with (
            tc.tile_pool(name="spool", bufs=1) as spool,
            tc.tile_pool(name="ppool", bufs=2, space="PSUM") as ppool,
        ):
            # ---- batch-lifetime staging tiles (all fp32) ----------------
            mstd_sb = spool.tile([MOT_M, BS], F32, tag="mstd")
            # values rows 0:3, loglik row 32
            vl_sb = spool.tile([33, BS], F32, tag="vl")
            act_sb = spool.tile([TOTAL_ACT, BS], F32, tag="act")   # tanh(raw)
            d2_sb = spool.tile([TOTAL_ACT, BS], F32, tag="d2")     # (raw-mean)^2
            u_sb = spool.tile([TOTAL_ACT, BS], F32, tag="u")       # tanh^2
            entp_sb = spool.tile([TOTAL_ACT, 2], F32, tag="entp")
            ones_sb = spool.tile([TOTAL_ACT, 1], F32, tag="ones")
            nc.vector.memset(ones_sb[:], 1.0)
            onesr_sb = spool.tile([TOTAL_ACT, 1], F32, tag="onesr")
            nc.vector.tensor_copy(onesr_sb[:].bitcast(mybir.dt.float32r), ones_sb[:])
            c_minstd = spool.tile([TOTAL_ACT, 1], F32, tag="cmin")
            nc.vector.memset(c_minstd[:], MIN_STD)
            c_eps = spool.tile([TOTAL_ACT, 1], F32, tag="ceps")
            nc.vector.memset(c_eps[:], 1.0 + 1e-6)

            def mm(out, lhsT, rhs, start, stop):
                nc.tensor.matmul(out, lhsT, rhs, start=start, stop=stop)

            # ================= phase A: network + motor ==================
            with (
                tc.tile_pool(name="wpool", bufs=1) as wpool,
                tc.tile_pool(name="obspool", bufs=2) as obspool,
                tc.tile_pool(name="rawpool", bufs=2) as rawpool,
                tc.tile_pool(name="xpool", bufs=2) as xpool,
                tc.tile_pool(name="tpool", bufs=2) as tpool,
                tc.tile_pool(name="rpool", bufs=1) as rpool,
                tc.tile_pool(name="hpool", bufs=1) as hpool,
            ):
                # ---- persistent weight tiles ----------------------------
                w_inp_sb = []
                w_e1_sb = []
                for ci in range(N_CHUNKS):
                    ti = wpool.tile([128, H], BF16, tag=f"winp{ci}")
                    nc.sync.dma_start(out=ti[:], in_=w_inp[ci * 128:(ci + 1) * 128, :])
                    w_inp_sb.append(ti)
                for ci in range(N_CHUNKS):
                    w_e1_sb.append(wpool.tile([128, 512], BF16, tag=f"we1{ci}",
                                              name=f"we1_{ci}"))
                w_msg_sb = wpool.tile([128, 20 * 512], BF16, tag="wmsg")
                w_mot_sb = wpool.tile([128, 22 * MOT_M], BF16, tag="wmot")
                w_e2_sb = wpool.tile([128, 2048], BF16, tag="we2")
                w_e3_sb = wpool.tile([128, 1024], BF16, tag="we3")
                w_hd_sb = wpool.tile([128, 6], BF16, tag="whd")
                b_mot_sb = wpool.tile([MOT_M, 1], F32, tag="bmot")
                b_hd_sb = wpool.tile([3, 1], F32, tag="bhd")
                if not zero_bias:
                    b_inp_sb = wpool.tile([128, 22], F32, tag="binp")
                    b_msg_sb = wpool.tile([128, 40], F32, tag="bmsg")
                    b_e1_sb = wpool.tile([128, 4], F32, tag="be1")
                    b_e2_sb = wpool.tile([128, 4], F32, tag="be2")
                    b_e3_sb = wpool.tile([128, 2], F32, tag="be3")

                def load_big_weights():
                    # issued after tile 0's obs DMAs so the first input tiles
                    # land immediately; these stream in behind them.
                    if not zero_bias:
                        nc.sync.dma_start(out=b_inp_sb[:], in_=b_inp[:])
                        nc.sync.dma_start(out=b_msg_sb[:], in_=b_msg[:])
                        nc.sync.dma_start(out=b_e1_sb[:], in_=b_e1[:])
                        nc.sync.dma_start(out=b_e2_sb[:], in_=b_e2[:])
                        nc.sync.dma_start(out=b_e3_sb[:], in_=b_e3[:])
                    for q in range(8):
                        nc.sync.dma_start(out=w_msg_sb[:, q * 1280:(q + 1) * 1280],
                                          in_=w_msg[:, q * 1280:(q + 1) * 1280])
                    for ci in range(N_CHUNKS):
                        te = w_e1_sb[ci]
                        nc.sync.dma_start(out=te[:],
                                          in_=w_e1[ci * 128:(ci + 1) * 128, :])
                    nc.sync.dma_start(out=w_mot_sb[:], in_=w_mot[:])
                    nc.sync.dma_start(out=w_e2_sb[:], in_=w_e2[:])
                    nc.sync.dma_start(out=w_e3_sb[:], in_=w_e3[:])
                    nc.sync.dma_start(out=w_hd_sb[:], in_=w_hd[:])
                    nc.sync.dma_start(out=b_mot_sb[:], in_=b_mot[:])
                    nc.sync.dma_start(out=b_hd_sb[:], in_=b_hd[:])

                for t in range(NTILES):
                    ct = t * NT

                    # xall: module k at cols [k*2*NT, (k+1)*2*NT); within a
                    # module: cols 0:NT = features 0:128, NT:2NT = 128:256
                    xall = xpool.tile([128, 11 * 2 * NT], BF16, tag="xall")

                    def xsl(k, kc=None):
                        base = MIDX[k] * 2 * NT
                        if kc is None:
                            return xall[:, base:base + 2 * NT]
                        return xall[:, base + kc * NT: base + (kc + 1) * NT]

                    def evict_swish(ps_ap, dst_ap, nblk, bias_cols):
                        if zero_bias:
                            nc.scalar.activation(dst_ap, ps_ap, AF.Silu)
                        else:
                            for i in range(nblk):
                                bt, bc = bias_cols[i]
                                nc.scalar.activation(
                                    dst_ap[:, i * NT:(i + 1) * NT],
                                    ps_ap[:, i * NT:(i + 1) * NT],
                                    AF.Silu, bias=bt[:, bc:bc + 1])

                    # -- load obs chunks + raw slice ----------------------
                    obs_t = []
                    for ci in range(N_CHUNKS):
                        ob = obspool.tile([128, NT], BF16, tag=f"obs{ci}")
                        nc.sync.dma_start(
                            out=ob[:],
                            in_=obsT[ci * 128:(ci + 1) * 128, ct:ct + NT])
                        obs_t.append(ob)
                    raw_t = rawpool.tile([TOTAL_ACT, NT], F32, tag="raw")
                    nc.sync.dma_start(out=raw_t[:], in_=rawT[:, ct:ct + NT])
                    if t == 0:
                        load_big_weights()
                    # tanh(raw) lives in the silu table set -> in-loop
                    nc.scalar.activation(act_sb[:, ct:ct + NT], raw_t[:], AF.Tanh)
                    nc.vector.tensor_mul(u_sb[:, ct:ct + NT],
                                         act_sb[:, ct:ct + NT],
                                         act_sb[:, ct:ct + NT])

                    # -- input layers: pairs of adjacent modules ----------
                    pairs = [("root", "torso"), ("head", "arm_L"),
                             ("arm_R", "hand_L"), ("hand_R", "leg_L"),
                             ("leg_R", "foot_L"), ("foot_R",)]
                    for pr in pairs:
                        pw = len(pr) * 2 * NT
                        ps = ppool.tile([128, 2048], F32, tag="ps")
                        for j, k in enumerate(pr):
                            ci, ro = MOD_CHUNK[k]
                            d = OBS[k]
                            for mc in range(2):
                                mm(ps[:, j * 2 * NT + mc * NT:
                                      j * 2 * NT + (mc + 1) * NT],
                                   w_inp_sb[ci][ro:ro + d, mc * 128:(mc + 1) * 128],
                                   obs_t[ci][ro:ro + d, :],
                                   start=True, stop=True)
                        base = MIDX[pr[0]] * 2 * NT
                        bias_cols = []
                        if not zero_bias:
                            for k in pr:
                                for mc in range(2):
                                    bias_cols.append((b_inp_sb, MIDX[k] * 2 + mc))
                        evict_swish(ps[:, 0:pw], xall[:, base:base + pw],
                                    len(pr) * 2, bias_cols)

                    # -- message-passing pair: swish(W.T @ src) -> tmp ----
                    def msg_pair(kind, mods, srcs):
                        ps = ppool.tile([128, 2048], F32, tag="ps")
                        for j, (k, src) in enumerate(zip(mods, srcs)):
                            mi = MSG_IDX[(kind, k)]
                            wb = mi * 512
                            for mc in range(2):
                                for kc in range(2):
                                    mm(ps[:, j * 2 * NT + mc * NT:
                                          j * 2 * NT + (mc + 1) * NT],
                                       w_msg_sb[:, wb + kc * 256 + mc * 128:
                                                wb + kc * 256 + (mc + 1) * 128],
                                       src[:, kc * NT:(kc + 1) * NT],
                                       start=(kc == 0), stop=(kc == 1))
                        tmp = tpool.tile([128, 2048], BF16, tag="msgtmp")
                        w = len(mods) * 2 * NT
                        bias_cols = []
                        if not zero_bias:
                            for k in mods:
                                mi = MSG_IDX[(kind, k)]
                                for mc in range(2):
                                    bias_cols.append((b_msg_sb, mi * 2 + mc))
                        evict_swish(ps[:, 0:w], tmp[:, 0:w], len(mods) * 2, bias_cols)
                        return tmp

                    # -- critic stages as closures, interleaved below -----
                    crit = {}

                    def enc1():
                        ps1 = ppool.tile([128, 2048], F32, tag="ps")
                        for mc in range(4):
                            for ci in range(N_CHUNKS):
                                mm(ps1[:, mc * NT:(mc + 1) * NT],
                                   w_e1_sb[ci][:, mc * 128:(mc + 1) * 128],
                                   obs_t[ci][:],
                                   start=(ci == 0), stop=(ci == N_CHUNKS - 1))
                        h1 = hpool.tile([128, 2048], BF16, tag="h1")
                        if zero_bias:
                            nc.scalar.activation(h1[:], ps1[:], AF.Silu)
                        else:
                            for mc in range(4):
                                nc.scalar.activation(h1[:, mc * NT:(mc + 1) * NT],
                                                     ps1[:, mc * NT:(mc + 1) * NT],
                                                     AF.Silu,
                                                     bias=b_e1_sb[:, mc:mc + 1])
                        crit["h1"] = h1

                    def enc2():
                        h1 = crit["h1"]
                        ps2 = ppool.tile([128, 2048], F32, tag="ps")
                        for mc in range(4):
                            for kc in range(4):
                                mm(ps2[:, mc * NT:(mc + 1) * NT],
                                   w_e2_sb[:, kc * 512 + mc * 128:
                                           kc * 512 + (mc + 1) * 128],
                                   h1[:, kc * NT:(kc + 1) * NT],
                                   start=(kc == 0), stop=(kc == 3))
                        h2 = hpool.tile([128, 2048], BF16, tag="h2")
                        if zero_bias:
                            nc.scalar.activation(h2[:], ps2[:], AF.Silu)
                        else:
                            for mc in range(4):
                                nc.scalar.activation(h2[:, mc * NT:(mc + 1) * NT],
                                                     ps2[:, mc * NT:(mc + 1) * NT],
                                                     AF.Silu,
                                                     bias=b_e2_sb[:, mc:mc + 1])
                        crit["h2"] = h2

                    def enc3():
                        h2 = crit["h2"]
                        ps3 = ppool.tile([128, 2048], F32, tag="ps")
                        for mc in range(2):
                            for kc in range(4):
                                mm(ps3[:, mc * NT:(mc + 1) * NT],
                                   w_e3_sb[:, kc * 256 + mc * 128:
                                           kc * 256 + (mc + 1) * 128],
                                   h2[:, kc * NT:(kc + 1) * NT],
                                   start=(kc == 0), stop=(kc == 3))
                        h3 = hpool.tile([128, 1024], BF16, tag="h3")
                        if zero_bias:
                            nc.scalar.activation(h3[:], ps3[:, 0:1024], AF.Silu)
                        else:
                            for mc in range(2):
                                nc.scalar.activation(h3[:, mc * NT:(mc + 1) * NT],
                                                     ps3[:, mc * NT:(mc + 1) * NT],
                                                     AF.Silu,
                                                     bias=b_e3_sb[:, mc:mc + 1])
                        crit["h3"] = h3

                    def heads():
                        h3 = crit["h3"]
                        psv = crit["psm"]  # share the motor psum tile, bank 1
                        for kc in range(2):
                            mm(psv[0:3, NT:2 * NT], w_hd_sb[:, kc * 3:(kc + 1) * 3],
                               h3[:, kc * NT:(kc + 1) * NT],
                               start=(kc == 0), stop=(kc == 1))
                        nc.vector.tensor_scalar_add(vl_sb[0:3, ct:ct + NT],
                                                    psv[0:3, NT:2 * NT],
                                                    b_hd_sb[0:3, 0:1])

                    enc1()

                    # afferent leaf -> mid (merged adds over adjacent dst)
                    tmp = msg_pair("aff", ["hand_L", "hand_R"],
                                   [xsl("hand_L"), xsl("hand_R")])
                    d0 = MIDX["arm_L"] * 2 * NT
                    nc.vector.tensor_add(xall[:, d0:d0 + 4 * NT],
                                         xall[:, d0:d0 + 4 * NT], tmp[:, 0:4 * NT])
                    tmp = msg_pair("aff", ["foot_L", "foot_R"],
                                   [xsl("foot_L"), xsl("foot_R")])
                    d0 = MIDX["leg_L"] * 2 * NT
                    nc.vector.tensor_add(xall[:, d0:d0 + 4 * NT],
                                         xall[:, d0:d0 + 4 * NT], tmp[:, 0:4 * NT])
                    # afferent mid -> root: 3 pairs, tree-reduced
                    rts = []
                    for pi, mods in enumerate((["arm_L", "arm_R"],
                                               ["leg_L", "leg_R"],
                                               ["torso", "head"])):
                        tmp = msg_pair("aff", mods, [xsl(mods[0]), xsl(mods[1])])
                        rt = rpool.tile([128, 2 * NT], BF16, tag=f"rt{pi}")
                        nc.vector.tensor_add(rt[:], tmp[:, 0:2 * NT],
                                             tmp[:, 2 * NT:4 * NT])
                        rts.append(rt)
                    nc.vector.tensor_add(rts[0][:], rts[0][:], rts[1][:])
                    nc.vector.tensor_add(rts[0][:], rts[0][:], rts[2][:])
                    nc.vector.tensor_add(xsl("root"), xsl("root"), rts[0][:])

                    enc2()

                    # efferent root wave
                    for mods in (["torso", "head"], ["arm_L", "arm_R"],
                                 ["leg_L", "leg_R"]):
                        tmp = msg_pair("eff", mods, [xsl("root"), xsl("root")])
                        d0 = MIDX[mods[0]] * 2 * NT
                        nc.vector.tensor_add(xall[:, d0:d0 + 4 * NT],
                                             xall[:, d0:d0 + 4 * NT],
                                             tmp[:, 0:4 * NT])
                    enc3()

                    # efferent leaf wave
                    tmp = msg_pair("eff", ["hand_L", "hand_R"],
                                   [xsl("arm_L"), xsl("arm_R")])
                    d0 = MIDX["hand_L"] * 2 * NT
                    nc.vector.tensor_add(xall[:, d0:d0 + 4 * NT],
                                         xall[:, d0:d0 + 4 * NT], tmp[:, 0:4 * NT])
                    tmp = msg_pair("eff", ["foot_L", "foot_R"],
                                   [xsl("leg_L"), xsl("leg_R")])
                    d0 = MIDX["foot_L"] * 2 * NT
                    nc.vector.tensor_add(xall[:, d0:d0 + 4 * NT],
                                         xall[:, d0:d0 + 4 * NT], tmp[:, 0:4 * NT])

                    # -- motor heads: 22 accumulating matmuls -> [108, NT] -
                    psm = ppool.tile([128, 2048], F32, tag="ps")
                    crit["psm"] = psm
                    nmm = 0
                    for k in MODULES:
                        for kc in range(2):
                            blk = (2 * MIDX[k] + kc) * MOT_M
                            mm(psm[0:MOT_M, 0:NT], w_mot_sb[:, blk:blk + MOT_M],
                               xsl(k, kc), start=(nmm == 0), stop=(nmm == 21))
                            nmm += 1
                    nc.vector.tensor_scalar_add(mstd_sb[:, ct:ct + NT],
                                                psm[0:MOT_M, 0:NT],
                                                b_mot_sb[0:MOT_M, 0:1])
                    # d2 = (raw - mean)^2, computed in-loop (fp32)
                    nc.vector.tensor_sub(d2_sb[:, ct:ct + NT], raw_t[:],
                                         mstd_sb[0:44, ct:ct + NT])
                    nc.vector.tensor_mul(d2_sb[:, ct:ct + NT],
                                         d2_sb[:, ct:ct + NT],
                                         d2_sb[:, ct:ct + NT])

                    heads()

            # ================= phase B: distribution math ================
            # Two column halves, software-pipelined so the serial exp/ln ACT
            # chain of one half overlaps the DVE work of the other.
            with tc.tile_pool(name="dpool", bufs=1) as dpool:
                HB = BS // 2
                bufA = [dpool.tile([TOTAL_ACT, HB], F32, tag=f"bufA{h}",
                                   name=f"bufA{h}") for h in range(2)]
                bufB = [dpool.tile([TOTAL_ACT, HB], F32, tag=f"bufB{h}",
                                   name=f"bufB{h}") for h in range(2)]
                bufC = [dpool.tile([TOTAL_ACT, HB], F32, tag=f"bufC{h}",
                                   name=f"bufC{h}") for h in range(2)]
                for h in range(2):
                    c0 = h * HB
                    mean = mstd_sb[0:44, c0:c0 + HB]
                    stdp = mstd_sb[64:64 + 44, c0:c0 + HB]
                    A, Bb, C = bufA[h], bufB[h], bufC[h]
                    nc.scalar.activation(A[:], stdp, AF.Exp)
                    nc.scalar.activation(Bb[:], A[:], AF.Ln, bias=1.0)
                    nc.scalar.activation(A[:], Bb[:], AF.Ln, bias=c_minstd[:],
                                         accum_out=entp_sb[:, h:h + 1])
                    nc.scalar.activation(Bb[:], A[:], AF.Exp, scale=-2.0)
                    nc.vector.tensor_mul(C[:].bitcast(mybir.dt.float32r),
                                         d2_sb[:, c0:c0 + HB], Bb[:])
                    nc.vector.scalar_tensor_tensor(Bb[:], C[:], -0.5, A[:],
                                                   op0=ALU.mult,
                                                   op1=ALU.subtract)
                    nc.scalar.activation(A[:], u_sb[:, c0:c0 + HB], AF.Ln,
                                         scale=-1.0, bias=c_eps[:])
                    # lp produced as f32r so the reduction matmul can run
                    # at 1 cycle/row (values only lose ~2^-13 mantissa)
                    nc.vector.tensor_sub(C[:].bitcast(mybir.dt.float32r),
                                         Bb[:], A[:])
                    pl = ppool.tile([128, 2048], F32, tag="ps")
                    for tt in range(NTILES // 2):
                        ct = tt * NT
                        nc.tensor.matmul(pl[0:1, ct:ct + NT],
                                         onesr_sb[:].bitcast(mybir.dt.float32r),
                                         C[:, ct:ct + NT].bitcast(mybir.dt.float32r),
                                         start=True, stop=True)
                    nc.vector.tensor_scalar_add(
                        vl_sb[32:33, c0:c0 + HB], pl[0:1, 0:2048],
                        float(-22.0 * LOG2PI))

                # ---- output DMAs ----------------------------------------
                nc.sync.dma_start(out=out_act[:], in_=act_sb[:])
                nc.sync.dma_start(out=out_ll[:], in_=vl_sb[32:33, :])
                nc.sync.dma_start(out=out_vals[:], in_=vl_sb[0:3, :])
                nc.sync.dma_start(out=out_ent[:], in_=entp_sb[:])

    nc.finalize()
    return nc


# ---------------------------------------------------------------------------
# Host-side weight packing
# ---------------------------------------------------------------------------

def _pack_msg(Ws):
    """20 [256,256] matrices -> [128, 20*512]; (kc,mc) block at j*512+kc*256+mc*128."""
    out = np.zeros((128, len(Ws) * 512), np.float32)
    for j, W in enumerate(Ws):
        blk = W.reshape(2, 128, 2, 128).transpose(1, 0, 2, 3).reshape(128, 512)
        out[:, j * 512:(j + 1) * 512] = blk
    return out.astype(BF)


def _pack_e2(W):  # [512, 512] -> [128, 4*512]
    return np.ascontiguousarray(
        W.reshape(4, 128, 512).transpose(1, 0, 2).reshape(128, 2048)).astype(BF)


def _pack_e3(W):  # [512, 256] -> [128, 4*256]
    return np.ascontiguousarray(
        W.reshape(4, 128, 256).transpose(1, 0, 2).reshape(128, 1024)).astype(BF)


def _pack_hd(W):  # [256, 3] -> [128, 2*3]
    out = np.zeros((128, 6), np.float32)
    for kc in range(2):
        out[:, kc * 3:(kc + 1) * 3] = W[kc * 128:(kc + 1) * 128, :]
    return out.astype(BF)


def _pack_mot(params):
    """Block-diagonal motor weights: [128, 22*108].

    K-chunk (module k, kc) at cols (2*MIDX[k]+kc)*108; within the 108 columns,
    module k's mean cols land at ACT_OFF[k].. and std cols at 64+ACT_OFF[k]..
    (row-64 start keeps the std block partition-base 32-aligned on chip).
    """
    out = np.zeros((128, 22 * MOT_M), np.float32)
    for k in MODULES:
        W = np.asarray(params["motor"][k][0], np.float32) * MOTOR_SCALE
        a = ACTD[k]
        for kc in range(2):
            blk = W[kc * 128:(kc + 1) * 128, :]
            base = (2 * MIDX[k] + kc) * MOT_M
            out[:, base + ACT_OFF[k]: base + ACT_OFF[k] + a] = blk[:, :a]
            out[:, base + 64 + ACT_OFF[k]: base + 64 + ACT_OFF[k] + a] = blk[:, a:]
    return out.astype(BF)


_NC_CACHE = {}
LAST_RESULT = None


def kernel(obs, raw_action, params):
    global LAST_RESULT
    obs = {k: np.asarray(v, np.float32) for k, v in obs.items()}
    raw = {k: np.asarray(v, np.float32) for k, v in raw_action.items()}

    def P(x):
        return np.asarray(x, np.float32)

    zero_bias = all([
        all(not P(params["inp"][k][1]).any() for k in MODULES),
        all(not P(params["aff"][k][1]).any() for k in NONROOT),
        all(not P(params["eff"][k][1]).any() for k in NONROOT),
        all(not P(p[1]).any() for p in params["enc"]),
    ])

    if zero_bias not in _NC_CACHE:
        _NC_CACHE[zero_bias] = _build_nc(zero_bias)
    nc = _NC_CACHE[zero_bias]

    # ---- pack host arrays ------------------------------------------------
    obsT = np.zeros((PAD_OBS, B), BF)
    for k in MODULES:
        ci, slot = MOD_CHUNK[k]
        obsT[ci * 128 + slot: ci * 128 + slot + OBS[k], :] = obs[k].T.astype(BF)
    rawT = np.ascontiguousarray(
        np.concatenate([raw[k] for k in MODULES], axis=1).T)        # [44, B]

    w_inp = np.zeros((PAD_OBS, H), np.float32)
    for k in MODULES:
        ci, slot = MOD_CHUNK[k]
        w_inp[ci * 128 + slot: ci * 128 + slot + OBS[k], :] = P(params["inp"][k][0])
    w_inp = w_inp.astype(BF)
    msg_mats = [P(params["aff"][k][0]) for k in NONROOT] + \
               [P(params["eff"][k][0]) for k in NONROOT]
    w_msg = _pack_msg(msg_mats)
    w_mot = _pack_mot(params)
    w_e1_orig = P(params["enc"][0][0])                               # [270, 512]
    w_e1 = np.zeros((PAD_OBS, 512), np.float32)
    for k in MODULES:
        ci, slot = MOD_CHUNK[k]
        w_e1[ci * 128 + slot: ci * 128 + slot + OBS[k], :] = \
            w_e1_orig[OBS_OFF[k]:OBS_OFF[k] + OBS[k], :]
    w_e1 = w_e1.astype(BF)
    w_e2 = _pack_e2(P(params["enc"][1][0]))
    w_e3 = _pack_e3(P(params["enc"][2][0]))
    w_hd = _pack_hd(np.concatenate([P(params["heads"][r][0]) for r in RK], axis=1))

    b_mot = np.zeros((MOT_M, 1), np.float32)
    for k in MODULES:
        bv = P(params["motor"][k][1]) * MOTOR_SCALE
        a = ACTD[k]
        b_mot[ACT_OFF[k]:ACT_OFF[k] + a, 0] = bv[:a]
        b_mot[64 + ACT_OFF[k]:64 + ACT_OFF[k] + a, 0] = bv[a:]
    b_hd = np.stack([P(params["heads"][r][1]) for r in RK]).reshape(3, 1)

    shared = {
        "w_inp": w_inp, "w_msg": w_msg, "w_mot": w_mot, "w_e1": w_e1,
        "w_e2": w_e2, "w_e3": w_e3, "w_hd": w_hd, "b_mot": b_mot, "b_hd": b_hd,
    }
    if not zero_bias:
        b_inp = np.zeros((128, 22), np.float32)
        for k in MODULES:
            bv = P(params["inp"][k][1])
            for mc in range(2):
                b_inp[:, MIDX[k] * 2 + mc] = bv[mc * 128:(mc + 1) * 128]
        b_msg = np.zeros((128, 40), np.float32)
        for kind in ("aff", "eff"):
            for k in NONROOT:
                mi = MSG_IDX[(kind, k)]
                bv = P(params[kind][k][1])
                for mc in range(2):
                    b_msg[:, mi * 2 + mc] = bv[mc * 128:(mc + 1) * 128]
        b_e1 = P(params["enc"][0][1]).reshape(4, 128).T.copy()
        b_e2 = P(params["enc"][1][1]).reshape(4, 128).T.copy()
        b_e3 = P(params["enc"][2][1]).reshape(2, 128).T.copy()
        shared.update({"b_inp": b_inp, "b_msg": b_msg, "b_e1": b_e1,
                       "b_e2": b_e2, "b_e3": b_e3})

    in_maps = []
    for i in range(N_CORES):
        m = dict(shared)
        m["obsT"] = np.ascontiguousarray(obsT[:, i * BS:(i + 1) * BS])
        m["rawT"] = np.ascontiguousarray(rawT[:, i * BS:(i + 1) * BS])
        in_maps.append(m)

    trace = bool(int(os.environ.get("KERNEL_TRACE", "0")))
    res = run_bass_kernel_spmd(nc, in_maps, list(range(N_CORES)), trace=trace)
    LAST_RESULT = res

    out = np.empty((B, 49), np.float32)
    ent_sum = 0.0
    for i in range(N_CORES):
        r = res.results[i]
        sl = slice(i * BS, (i + 1) * BS)
        out[sl, 0:44] = r["out_act"].T
        out[sl, 44] = r["out_ll"][0]
        out[sl, 45:48] = r["out_vals"].T
        ent_sum += float(r["out_ent"].sum())
    ent = 22.0 * (1.0 + LOG2PI) + ent_sum / B
    out[:, 48] = -ENT_W * ent
    return out
